# revision 1
# baseline (speedup 1.0000x reference)
"""Trainium2 Bass kernel for the DWA middle layer (moe_routing).

Math (factored form of the reference):
    t     = h_A @ V_flat^T                      # [B, N*R]
    s     = t * repeat(alpha, R, axis=1)        # [B, N*R]
    h_T   = s @ U_flat^T + h_A @ W_base^T + [alpha, 1] @ [bias_pool; b_base]
    out   = LayerNorm(h_A + gamma * h_T) * ln_scale + ln_bias

Sharding: data-parallel over the batch dim (32 rows per core, 8 cores).
Weight matrices are replicated; on the host we only re-lay them out
(transpose/reshape/concat into the SBUF-native partition-major layout)
so the contraction dim lands on SBUF partitions — all arithmetic runs
on device.

All PE matmuls keep the (small) activations stationary and stream the
weight matrices as the moving operand at N=512.  Weight DMAs are issued
in 512KB k-tile chunks interleaved with the matmuls that consume them,
so the PE pipeline runs under the (HBM-bound) weight stream.
"""

import os
from contextlib import ExitStack

import numpy as np

import concourse.bacc as bacc
import concourse.mybir as mybir
import concourse.tile as tile
from concourse import bass_utils, masks

F32 = mybir.dt.float32
F32R = mybir.dt.float32r

D = 1024          # d_A == d_B
B_CORE = 32       # batch rows per core
N_EXP = 64        # experts
R_RANK = 16       # rank per expert
N_CORES = 8
KT = D // 128     # 8 contraction tiles of 128
NH = D // 512     # 2 moving halves of 512

# "f32r" = raw-fp32 single-pass PE mode (faster, slightly relaxed
# multiply precision); "f32" = full two-pass fp32.
MATMUL_MODE = os.environ.get("DWA_MATMUL_MODE", "f32r")
STAGE = os.environ.get("DWA_STAGE", "full")

_COMPILED = {}


def _build(mode, stage="full"):
    nc = bacc.Bacc("TRN2", debug=False, num_devices=N_CORES,
                   enable_partition_id=False)
    WDT = F32R if mode == "f32r" else F32

    ha_d = nc.dram_tensor("ha", [B_CORE, D], F32, kind="ExternalInput")
    al_d = nc.dram_tensor("al", [B_CORE, N_EXP], F32, kind="ExternalInput")
    # weights in SBUF-native partition-major layout [128, KT*1024]
    vt_d = nc.dram_tensor("vt", [128, KT * D], WDT, kind="ExternalInput")
    ut_d = nc.dram_tensor("ut", [128, KT * D], WDT, kind="ExternalInput")
    wt_d = nc.dram_tensor("wt", [128, KT * D], WDT, kind="ExternalInput")
    bp_d = nc.dram_tensor("bp", [N_EXP + 1, D], WDT, kind="ExternalInput")
    lns_d = nc.dram_tensor("lns", [1, D], F32, kind="ExternalInput")
    lnb_d = nc.dram_tensor("lnb", [1, D], F32, kind="ExternalInput")
    gm_d = nc.dram_tensor("gm", [1, 1], F32, kind="ExternalInput")
    out_d = nc.dram_tensor("out", [B_CORE, D], F32, kind="ExternalOutput")

    with ExitStack() as ctx:
        tc = ctx.enter_context(tile.TileContext(nc))
        _emit(ctx, tc, WDT, stage, ha_d, al_d, vt_d, ut_d, wt_d, bp_d,
              lns_d, lnb_d, gm_d, out_d)

    nc.compile()
    return nc


def _emit(ctx, tc, WDT, stage, ha_d, al_d, vt_d, ut_d, wt_d, bp_d,
          lns_d, lnb_d, gm_d, out_d):
    nc = tc.nc
    MULT = mybir.AluOpType.mult
    ADD = mybir.AluOpType.add
    SQRT = mybir.ActivationFunctionType.Sqrt

    wpool = ctx.enter_context(tc.tile_pool(name="weights", bufs=1))
    sm = ctx.enter_context(tc.tile_pool(name="small", bufs=1))
    trp = ctx.enter_context(tc.tile_pool(name="trps", bufs=2, space="PSUM"))
    acc = ctx.enter_context(tc.tile_pool(name="acc", bufs=1, space="PSUM"))

    vt_sb = wpool.tile([128, KT * D], WDT, tag="vt")
    ut_sb = wpool.tile([128, KT * D], WDT, tag="ut")
    wt_sb = wpool.tile([128, KT * D], WDT, tag="wt")

    ha_sb = sm.tile([B_CORE, D], F32, tag="ha")
    al_sb = sm.tile([B_CORE, N_EXP + 1], F32, tag="al")  # [alpha | 1]
    bp_sb = sm.tile([N_EXP + 1, D], WDT, tag="bp")
    ident = sm.tile([128, 128], F32, tag="ident")
    x_sb = sm.tile([128, KT * B_CORE], WDT, tag="x")      # h_A^T tiles
    alt_sb = sm.tile([N_EXP + 1, B_CORE], WDT, tag="alt")  # [alpha^T; 1]
    s_sb = sm.tile([B_CORE, D], F32, tag="s")
    st_sb = sm.tile([128, KT * B_CORE], WDT, tag="st")    # s^T tiles
    hpre_sb = sm.tile([B_CORE, D], F32, tag="hpre")
    sq_sb = sm.tile([B_CORE, D], F32, tag="sq")
    y_sb = sm.tile([B_CORE, D], F32, tag="y")
    t2_sb = sm.tile([B_CORE, D], F32, tag="t2")
    out_sb = sm.tile([B_CORE, D], F32, tag="out")
    lnsr_sb = sm.tile([B_CORE, D], F32, tag="lnsr")
    lnbr_sb = sm.tile([B_CORE, D], F32, tag="lnbr")
    gmc_sb = sm.tile([B_CORE, 1], F32, tag="gmc")
    sum_h = [sm.tile([B_CORE, 1], F32, tag=f"sumh{h}", name=f"sumh{h}")
             for h in range(NH)]
    ssq_h = [sm.tile([B_CORE, 1], F32, tag=f"ssqh{h}", name=f"ssqh{h}")
             for h in range(NH)]
    sum_c = sm.tile([B_CORE, 1], F32, tag="sumc")
    m_c = sm.tile([B_CORE, 1], F32, tag="mc")
    msq_c = sm.tile([B_CORE, 1], F32, tag="msqc")
    ssq_c = sm.tile([B_CORE, 1], F32, tag="ssqc")
    var_c = sm.tile([B_CORE, 1], F32, tag="varc")
    std_c = sm.tile([B_CORE, 1], F32, tag="stdc")
    istd_c = sm.tile([B_CORE, 1], F32, tag="istdc")
    nmi_c = sm.tile([B_CORE, 1], F32, tag="nmic")
    eps_c = sm.tile([B_CORE, 1], F32, tag="epsc")
    warm_c = sm.tile([B_CORE, 1], F32, tag="warmc")

    # ---- activation loads first, then the weight chunk stream ----
    nc.sync.dma_start(out=ha_sb[:], in_=ha_d.ap())
    nc.sync.dma_start(out=al_sb[:, :N_EXP], in_=al_d.ap())
    dma_engs = (nc.sync, nc.scalar)
    # stream order matches consumption: t (vt) -> base (wt) -> delta (ut);
    # the final ut megabyte is split across both rings so the last-arriving
    # chunk is small
    chunks = []
    for w_sb, w_d in ((vt_sb, vt_d), (wt_sb, wt_d)):
        for i in range(0, KT, 2):
            chunks.append((w_sb, w_d, slice(D * i, D * (i + 2))))
    for i in range(0, KT - 2, 2):
        chunks.append((ut_sb, ut_d, slice(D * i, D * (i + 2))))
    chunks.append((ut_sb, ut_d, slice(D * (KT - 2), D * (KT - 1))))
    chunks.append((ut_sb, ut_d, slice(D * (KT - 1), D * KT)))
    for q, (w_sb, w_d, csl) in enumerate(chunks):
        dma_engs[q % 2].dma_start(out=w_sb[:, csl], in_=w_d.ap()[:, csl])
    # gamma/bias_pool (needed mid-kernel) ride the parallel SWDGE ring;
    # LN vectors (tail-only) go last on the HWDGE rings
    nc.gpsimd.dma_start(out=gmc_sb[:], in_=gm_d.ap().broadcast_to([B_CORE, 1]))
    nc.gpsimd.dma_start(out=bp_sb[:], in_=bp_d.ap())
    nc.sync.dma_start(out=lnsr_sb[:], in_=lns_d.ap().broadcast_to([B_CORE, D]))
    nc.scalar.dma_start(out=lnbr_sb[:], in_=lnb_d.ap().broadcast_to([B_CORE, D]))

    nc.vector.memset(al_sb[:, N_EXP:N_EXP + 1], 1.0)
    nc.vector.memset(eps_c[:], 1e-5)
    masks.make_identity(nc, ident[:])
    # preload both ACT tables (Square, Sqrt) off the critical path
    nc.scalar.activation(warm_c[:], eps_c[:],
                         mybir.ActivationFunctionType.Square)
    nc.scalar.activation(warm_c[:], eps_c[:], SQRT, bias=eps_c[:], scale=1.0)

    if stage == "loads":
        nc.vector.tensor_copy(out_sb[:], ha_sb[:])
        nc.sync.dma_start(out=out_d.ap(), in_=out_sb[:])
        return

    # ---- transposes: X = h_A^T (per 128-wide a-tile), [alpha^T; 1] ----
    for i in range(KT):
        tp = trp.tile([128, B_CORE], F32, tag="tr", name=f"trx{i}")
        nc.tensor.transpose(tp[:], ha_sb[:, 128 * i:128 * (i + 1)],
                            ident[:B_CORE, :B_CORE])
        nc.vector.tensor_copy(x_sb[:, B_CORE * i:B_CORE * (i + 1)], tp[:])
    tp = trp.tile([128, B_CORE], F32, tag="tr", name="tral")
    nc.tensor.transpose(tp[:N_EXP + 1], al_sb[:], ident[:B_CORE, :B_CORE])
    nc.vector.tensor_copy(alt_sb[:], tp[:N_EXP + 1])

    # ---- t = h_A @ V^T ; s = t * repeat(alpha, R) ----
    t_ps = [acc.tile([B_CORE, 512], F32, tag=f"t{h}", name=f"t_ps{h}")
            for h in range(NH)]
    for i in range(KT):
        for h in range(NH):
            nc.tensor.matmul(
                t_ps[h][:],
                x_sb[:, B_CORE * i:B_CORE * (i + 1)],
                vt_sb[:, D * i + 512 * h:D * i + 512 * (h + 1)],
                start=(i == 0), stop=(i == KT - 1),
            )
    for h in range(NH):
        o3 = s_sb[:, 512 * h:512 * (h + 1)].rearrange(
            "p (n r) -> p n r", r=R_RANK)
        i3 = t_ps[h][:].rearrange("p (n r) -> p n r", r=R_RANK)
        a3 = al_sb[:, 32 * h:32 * (h + 1)].unsqueeze(-1).broadcast_to(
            [B_CORE, 32, R_RANK])
        nc.vector.tensor_mul(o3, i3, a3)

    if stage == "t":
        nc.sync.dma_start(out=out_d.ap(), in_=s_sb[:])
        return

    # ---- s^T tiles ----
    for j in range(KT):
        tp = trp.tile([128, B_CORE], F32, tag="tr", name=f"trs{j}")
        nc.tensor.transpose(tp[:], s_sb[:, 128 * j:128 * (j + 1)],
                            ident[:B_CORE, :B_CORE])
        nc.vector.tensor_copy(st_sb[:, B_CORE * j:B_CORE * (j + 1)], tp[:])

    # ---- h_T = [alpha,1] @ [bias_pool; b_base] + h_A @ W^T + s @ U^T ----
    h_ps = [acc.tile([B_CORE, 512], F32, tag=f"h{h}", name=f"h_ps{h}")
            for h in range(NH)]
    for h in range(NH):
        nc.tensor.matmul(h_ps[h][:], alt_sb[:],
                         bp_sb[:, 512 * h:512 * (h + 1)],
                         start=True, stop=False)
    for i in range(KT):
        for h in range(NH):
            nc.tensor.matmul(
                h_ps[h][:],
                x_sb[:, B_CORE * i:B_CORE * (i + 1)],
                wt_sb[:, D * i + 512 * h:D * i + 512 * (h + 1)],
                start=False, stop=False,
            )
    for j in range(KT):
        for h in range(NH):
            nc.tensor.matmul(
                h_ps[h][:],
                st_sb[:, B_CORE * j:B_CORE * (j + 1)],
                ut_sb[:, D * j + 512 * h:D * j + 512 * (h + 1)],
                start=False, stop=(j == KT - 1),
            )
    for h in range(NH):
        sl = slice(512 * h, 512 * (h + 1))
        # h_pre = gamma * h_T + h_A, with row-sums for the mean
        nc.vector.scalar_tensor_tensor(
            out=hpre_sb[:, sl], in0=h_ps[h][:], scalar=gmc_sb[:],
            in1=ha_sb[:, sl], op0=MULT, op1=ADD,
            accum_out=sum_h[h][:])
        # row-sums of squares on the Scalar engine (parallel to DVE)
        nc.scalar.activation(sq_sb[:, sl], hpre_sb[:, sl],
                             mybir.ActivationFunctionType.Square,
                             accum_out=ssq_h[h][:])

    if stage == "h":
        nc.sync.dma_start(out=out_d.ap(), in_=hpre_sb[:])
        return

    # ---- LayerNorm via E[x^2] - E[x]^2 ----
    nc.vector.tensor_add(sum_c[:], sum_h[0][:], sum_h[1][:])
    nc.vector.tensor_add(ssq_c[:], ssq_h[0][:], ssq_h[1][:])
    nc.scalar.mul(m_c[:], sum_c[:], 1.0 / D)
    nc.vector.tensor_mul(msq_c[:], m_c[:], m_c[:])
    nc.vector.scalar_tensor_tensor(
        out=var_c[:], in0=ssq_c[:], scalar=1.0 / D, in1=msq_c[:],
        op0=MULT, op1=mybir.AluOpType.subtract)
    nc.scalar.activation(std_c[:], var_c[:], SQRT, bias=eps_c[:], scale=1.0)
    nc.vector.reciprocal(istd_c[:], std_c[:])
    # out = hpre*istd*lns + (lnb - m*istd*lns), in halves overlapped
    # with the output DMA
    nc.vector.tensor_mul(nmi_c[:], m_c[:], istd_c[:])
    nc.scalar.mul(nmi_c[:], nmi_c[:], -1.0)
    for h in range(NH):
        sl = slice(512 * h, 512 * (h + 1))
        nc.vector.scalar_tensor_tensor(
            out=t2_sb[:, sl], in0=lnsr_sb[:, sl], scalar=nmi_c[:],
            in1=lnbr_sb[:, sl], op0=MULT, op1=ADD)
        nc.vector.scalar_tensor_tensor(
            out=y_sb[:, sl], in0=hpre_sb[:, sl], scalar=istd_c[:],
            in1=lnsr_sb[:, sl], op0=MULT, op1=MULT)
        nc.vector.tensor_add(out_sb[:, sl], y_sb[:, sl], t2_sb[:, sl])
        nc.sync.dma_start(out=out_d.ap()[:, sl], in_=out_sb[:, sl])


def _to_sbuf_layout(a):
    """[KT*128, D] logical -> [128, KT*D] partition-major."""
    return np.ascontiguousarray(
        a.reshape(KT, 128, D).transpose(1, 0, 2).reshape(128, KT * D))


def _prep_in_maps(inputs):
    def f32c(x):
        return np.ascontiguousarray(np.asarray(x, dtype=np.float32))

    h_a = f32c(inputs["h_A"])
    alpha = f32c(inputs["alpha"])
    pool = np.asarray(inputs["pool_vectors"], dtype=np.float32)
    w_base = np.asarray(inputs["W_base"], dtype=np.float32)

    # pool_vectors rows: [U_n (D*R) | V_n (R*D) | bias_n (D)]
    u = pool[:, :D * R_RANK].reshape(N_EXP, D, R_RANK)
    v = pool[:, D * R_RANK:2 * D * R_RANK].reshape(N_EXP, R_RANK, D)
    bias_pool = pool[:, 2 * D * R_RANK:]                    # [64, D]
    bb = np.asarray(inputs["b_base"], dtype=np.float32).reshape(1, D)
    bp = f32c(np.concatenate([bias_pool, bb], axis=0))      # [65, D]
    ut = _to_sbuf_layout(
        f32c(u.transpose(0, 2, 1).reshape(N_EXP * R_RANK, D)))  # [(n,r), c]
    vt = _to_sbuf_layout(f32c(v.reshape(N_EXP * R_RANK, D).T))  # [a, (n,r)]
    wt = _to_sbuf_layout(f32c(w_base.T))                        # [a, c]
    lns = f32c(inputs["ln_scale"]).reshape(1, D)
    lnb = f32c(inputs["ln_bias"]).reshape(1, D)
    gm = f32c(inputs["gamma"]).reshape(1, 1)

    in_maps = []
    for k in range(N_CORES):
        rows = slice(B_CORE * k, B_CORE * (k + 1))
        in_maps.append({
            "ha": f32c(h_a[rows]), "al": f32c(alpha[rows]),
            "vt": vt, "ut": ut, "wt": wt, "bp": bp,
            "lns": lns, "lnb": lnb, "gm": gm,
        })
    return in_maps


def get_compiled(mode=None, stage=None):
    key = (mode or MATMUL_MODE, stage or STAGE)
    if key not in _COMPILED:
        _COMPILED[key] = _build(*key)
    return _COMPILED[key]


def kernel(**inputs):
    nc = get_compiled()
    in_maps = _prep_in_maps(inputs)
    res = bass_utils.run_bass_kernel_spmd(
        nc, in_maps, core_ids=list(range(N_CORES)))
    return np.concatenate([r["out"] for r in res.results], axis=0)



# revision 4
# speedup vs baseline: 1.4920x; 1.4920x over previous
"""Trainium2 Bass kernel for the DWA middle layer (moe_routing).

Math (factored form of the reference; W_assembled is never materialized):
    t     = h_A @ V_flat^T                      # [B, N*R]
    s     = t * repeat(alpha, R, axis=1)        # [B, N*R]
    h_T   = s @ U_flat^T + h_A @ W_base^T + [alpha, 1] @ [bias_pool; b_base]
    out   = LayerNorm(h_A + gamma * h_T) * ln_scale + ln_bias

Sharding: data-parallel over the batch dim (32 rows per core, 8 cores);
weights replicated.  The kernel is HBM-bound on the three 1024x1024
weight streams, so they are sent as fp8 e4m3 (scaled x32 on the host;
the scale is folded into alpha, the bias matrix and gamma, so the
device math is unchanged up to fp8 rounding — measured end-to-end
rel-err ~1.7e-3 against the fp32 reference, well inside the 2e-2 gate).
Matmuls run in DoubleRow fp8 mode (256-deep contraction per pass).

Host-side prep only re-lays-out / scales / casts inputs (transpose,
reshape, concat, dtype cast) — all arithmetic between tensors runs on
device.

All PE matmuls keep the small per-core activations stationary and
stream the weight matrices as the moving operand.  Weight DMAs are
issued as 256KB chunks alternating over both HWDGE rings in
consumption order (V -> W_base -> U, low output-half first) so the
PE/DVE pipeline runs under the HBM weight stream and the epilogue for
the first output half starts while the last U chunks are in flight.
"""

import os
from contextlib import ExitStack

import ml_dtypes
import numpy as np

import concourse.bacc as bacc
import concourse.mybir as mybir
import concourse.tile as tile
from concourse import bass_utils, masks

F32 = mybir.dt.float32
BF16 = mybir.dt.bfloat16
F8 = mybir.dt.float8e4
NP_F8 = ml_dtypes.float8_e4m3
NP_BF16 = ml_dtypes.bfloat16

D = 1024          # d_A == d_B
B_CORE = 32       # batch rows per core
N_EXP = 64        # experts
R_RANK = 16       # rank per expert
N_CORES = 8
KT = 8            # 128-deep contraction tiles
JT = 4            # DoubleRow 256-deep contraction tiles
NH = 2            # output halves of 512
WSC = 32.0        # fp8 weight scale (folded into alpha/bias/gamma)

_COMPILED = {}


def _build(general_ln):
    nc = bacc.Bacc("TRN2", debug=False, num_devices=N_CORES,
                   enable_partition_id=False)

    ha_d = nc.dram_tensor("ha", [B_CORE, D], F32, kind="ExternalInput")
    al_d = nc.dram_tensor("al", [B_CORE, N_EXP], F32, kind="ExternalInput")
    x8_d = nc.dram_tensor("x8", [128, JT * 2 * B_CORE], F8,
                          kind="ExternalInput")
    alt_d = nc.dram_tensor("alt", [N_EXP + 1, B_CORE], BF16,
                           kind="ExternalInput")
    # weights in DoubleRow-ready layout [128, (h j i n)] (see _dr_layout)
    vt_d = nc.dram_tensor("vt", [128, KT * D], F8, kind="ExternalInput")
    wt_d = nc.dram_tensor("wt", [128, KT * D], F8, kind="ExternalInput")
    ut_d = nc.dram_tensor("ut", [128, KT * D], F8, kind="ExternalInput")
    bp_d = nc.dram_tensor("bp", [N_EXP + 1, D], BF16, kind="ExternalInput")
    gm_d = nc.dram_tensor("gm", [1, 1], F32, kind="ExternalInput")
    if general_ln:
        lns_d = nc.dram_tensor("lns", [1, D], F32, kind="ExternalInput")
        lnb_d = nc.dram_tensor("lnb", [1, D], F32, kind="ExternalInput")
    else:
        lns_d = lnb_d = None
    out_d = nc.dram_tensor("out", [B_CORE, D], F32, kind="ExternalOutput")

    with ExitStack() as ctx:
        tc = ctx.enter_context(tile.TileContext(nc))
        _emit(ctx, tc, general_ln, ha_d, al_d, x8_d, alt_d, vt_d, wt_d,
              ut_d, bp_d, gm_d, lns_d, lnb_d, out_d)

    nc.compile()
    return nc


def _emit(ctx, tc, general_ln, ha_d, al_d, x8_d, alt_d, vt_d, wt_d, ut_d,
          bp_d, gm_d, lns_d, lnb_d, out_d):
    nc = tc.nc
    MULT = mybir.AluOpType.mult
    ADD = mybir.AluOpType.add
    SQRT = mybir.ActivationFunctionType.Sqrt
    DR = mybir.MatmulPerfMode.DoubleRow

    wpool = ctx.enter_context(tc.tile_pool(name="weights", bufs=1))
    sm = ctx.enter_context(tc.tile_pool(name="small", bufs=1))
    trp = ctx.enter_context(tc.tile_pool(name="trps", bufs=2, space="PSUM"))
    acc = ctx.enter_context(tc.tile_pool(name="acc", bufs=1, space="PSUM"))

    vt_sb = wpool.tile([128, KT * D], F8, tag="vt")
    wt_sb = wpool.tile([128, KT * D], F8, tag="wt")
    ut_sb = wpool.tile([128, KT * D], F8, tag="ut")

    ha_sb = sm.tile([B_CORE, D], F32, tag="ha")
    al_sb = sm.tile([B_CORE, N_EXP], F32, tag="al")
    x8_sb = sm.tile([128, JT * 2 * B_CORE], F8, tag="x8")
    alt_sb = sm.tile([N_EXP + 1, B_CORE], BF16, tag="alt")
    bp_sb = sm.tile([N_EXP + 1, D], BF16, tag="bp")
    ident = sm.tile([B_CORE, B_CORE], F32, tag="ident")
    s_sb = sm.tile([B_CORE, D], F32, tag="s")
    st_sb = sm.tile([128, KT * B_CORE], F8, tag="st")    # s^T tiles
    hpre_sb = sm.tile([B_CORE, D], F32, tag="hpre")
    sq_sb = sm.tile([B_CORE, D], F32, tag="sq")
    out_sb = sm.tile([B_CORE, D], F32, tag="out")
    gmc_sb = sm.tile([B_CORE, 1], F32, tag="gmc")
    sum_h = [sm.tile([B_CORE, 1], F32, tag=f"sumh{h}", name=f"sumh{h}")
             for h in range(NH)]
    ssq_h = [sm.tile([B_CORE, 1], F32, tag=f"ssqh{h}", name=f"ssqh{h}")
             for h in range(NH)]
    sum_c = sm.tile([B_CORE, 1], F32, tag="sumc")
    m_c = sm.tile([B_CORE, 1], F32, tag="mc")
    msq_c = sm.tile([B_CORE, 1], F32, tag="msqc")
    ssq_c = sm.tile([B_CORE, 1], F32, tag="ssqc")
    var_c = sm.tile([B_CORE, 1], F32, tag="varc")
    std_c = sm.tile([B_CORE, 1], F32, tag="stdc")
    istd_c = sm.tile([B_CORE, 1], F32, tag="istdc")
    nmi_c = sm.tile([B_CORE, 1], F32, tag="nmic")
    eps_c = sm.tile([B_CORE, 1], F32, tag="epsc")
    warm_c = sm.tile([B_CORE, 1], F32, tag="warmc")
    if general_ln:
        lnsr_sb = sm.tile([B_CORE, D], F32, tag="lnsr")
        lnbr_sb = sm.tile([B_CORE, D], F32, tag="lnbr")
        y_sb = sm.tile([B_CORE, D], F32, tag="y")
        t2_sb = sm.tile([B_CORE, D], F32, tag="t2")

    # ---- weight chunk stream: V -> W -> U, half-0 columns first, each
    # matrix's halves split across both HWDGE rings ----
    CH = KT * D // 4  # 2048 fp8 columns = 256KB per chunk
    for w_sb, w_d in ((vt_sb, vt_d), (wt_sb, wt_d), (ut_sb, ut_d)):
        for q in range(4):
            eng = (nc.sync, nc.scalar)[q % 2]
            csl = slice(CH * q, CH * (q + 1))
            eng.dma_start(out=w_sb[:, csl], in_=w_d.ap()[:, csl])

    # small tensors ride the parallel SWDGE ring
    nc.gpsimd.dma_start(out=x8_sb[:], in_=x8_d.ap())
    nc.gpsimd.dma_start(out=al_sb[:], in_=al_d.ap())
    nc.gpsimd.dma_start(out=alt_sb[:], in_=alt_d.ap())
    nc.gpsimd.dma_start(out=ha_sb[:], in_=ha_d.ap())
    nc.gpsimd.dma_start(out=bp_sb[:], in_=bp_d.ap())
    nc.gpsimd.dma_start(out=gmc_sb[:], in_=gm_d.ap().broadcast_to([B_CORE, 1]))
    if general_ln:
        nc.gpsimd.dma_start(out=lnsr_sb[:],
                            in_=lns_d.ap().broadcast_to([B_CORE, D]))
        nc.gpsimd.dma_start(out=lnbr_sb[:],
                            in_=lnb_d.ap().broadcast_to([B_CORE, D]))

    nc.vector.memset(eps_c[:], 1e-5)
    masks.make_identity(nc, ident[:])
    # preload both ACT tables (Square, Sqrt) off the critical path
    nc.scalar.activation(warm_c[:], eps_c[:],
                         mybir.ActivationFunctionType.Square)
    nc.scalar.activation(warm_c[:], eps_c[:], SQRT, bias=eps_c[:], scale=1.0)

    def dr_rhs(w_sb, h, j):
        # moving operand [128, 2, 512] for output half h, 256-ktile j
        base = (2 * h + 1) * 0  # layout is [h, j, i, n]
        off = h * (JT * D) + j * D
        return w_sb[:, off:off + D].rearrange("p (two n) -> p two n", two=2)

    def dr_lhs(x_sb, j):
        off = j * 2 * B_CORE
        return x_sb[:, off:off + 2 * B_CORE].rearrange(
            "p (two m) -> p two m", two=2)

    # ---- t = h_A @ V^T ; s8 = t * repeat(alpha/32, R)  (fp8 cast) ----
    t_ps = [acc.tile([B_CORE, 512], F32, tag=f"t{h}", name=f"t_ps{h}")
            for h in range(NH)]
    h_ps = [acc.tile([B_CORE, 512], F32, tag=f"h{h}", name=f"h_ps{h}")
            for h in range(NH)]

    for h in range(NH):
        for j in range(JT):
            nc.tensor.matmul(t_ps[h][:], dr_lhs(x8_sb, j), dr_rhs(vt_sb, h, j),
                             start=(j == 0), stop=(j == JT - 1), perf_mode=DR)
        # s = t * alpha_rep (alpha pre-divided by 32 on host)
        o3 = s_sb[:, 512 * h:512 * (h + 1)].rearrange(
            "p (n r) -> p n r", r=R_RANK)
        i3 = t_ps[h][:].rearrange("p (n r) -> p n r", r=R_RANK)
        a3 = al_sb[:, 32 * h:32 * (h + 1)].unsqueeze(-1).broadcast_to(
            [B_CORE, 32, R_RANK])
        nc.vector.tensor_mul(o3, i3, a3)
        # s^T tiles for this half (PE transpose f32, fp8 cast in the copy)
        for kk in range(4):
            k = 4 * h + kk
            tp = trp.tile([128, B_CORE], F32, tag="tr", name=f"trs{k}")
            nc.tensor.transpose(tp[:], s_sb[:, 128 * k:128 * (k + 1)],
                                ident[:])
            if kk % 2 == 0:
                nc.vector.tensor_copy(
                    st_sb[:, B_CORE * k:B_CORE * (k + 1)], tp[:])
            else:
                nc.scalar.activation(
                    st_sb[:, B_CORE * k:B_CORE * (k + 1)], tp[:],
                    mybir.ActivationFunctionType.Copy)

    # ---- h_T*32 = [al,1]@bp + h_A @ (32W)^T + s @ (32U)^T ----
    for h in range(NH):
        nc.tensor.matmul(h_ps[h][:], alt_sb[:],
                         bp_sb[:, 512 * h:512 * (h + 1)],
                         start=True, stop=False)
    for h in range(NH):
        for j in range(JT):
            nc.tensor.matmul(h_ps[h][:], dr_lhs(x8_sb, j), dr_rhs(wt_sb, h, j),
                             start=False, stop=False, perf_mode=DR)
    for h in range(NH):
        for j in range(JT):
            nc.tensor.matmul(h_ps[h][:], dr_lhs(st_sb, j), dr_rhs(ut_sb, h, j),
                             start=False, stop=(j == JT - 1), perf_mode=DR)
        sl = slice(512 * h, 512 * (h + 1))
        # h_pre = (gamma/32) * (32 h_T) + h_A, with row-sums for the mean
        nc.vector.scalar_tensor_tensor(
            out=hpre_sb[:, sl], in0=h_ps[h][:], scalar=gmc_sb[:],
            in1=ha_sb[:, sl], op0=MULT, op1=ADD,
            accum_out=sum_h[h][:])
        # row-sums of squares on the Scalar engine (parallel to DVE)
        nc.scalar.activation(sq_sb[:, sl], hpre_sb[:, sl],
                             mybir.ActivationFunctionType.Square,
                             accum_out=ssq_h[h][:])

    # ---- LayerNorm via E[x^2] - E[x]^2 ----
    nc.vector.tensor_add(sum_c[:], sum_h[0][:], sum_h[1][:])
    nc.vector.tensor_add(ssq_c[:], ssq_h[0][:], ssq_h[1][:])
    nc.scalar.mul(m_c[:], sum_c[:], 1.0 / D)
    nc.vector.tensor_mul(msq_c[:], m_c[:], m_c[:])
    nc.vector.scalar_tensor_tensor(
        out=var_c[:], in0=ssq_c[:], scalar=1.0 / D, in1=msq_c[:],
        op0=MULT, op1=mybir.AluOpType.subtract)
    nc.scalar.activation(std_c[:], var_c[:], SQRT, bias=eps_c[:], scale=1.0)
    nc.vector.reciprocal(istd_c[:], std_c[:])
    nc.vector.tensor_mul(nmi_c[:], m_c[:], istd_c[:])
    nc.scalar.mul(nmi_c[:], nmi_c[:], -1.0)

    for h in range(NH):
        sl = slice(512 * h, 512 * (h + 1))
        if general_ln:
            # out = hpre*istd*lns + (lnb - m*istd*lns)
            nc.vector.scalar_tensor_tensor(
                out=t2_sb[:, sl], in0=lnsr_sb[:, sl], scalar=nmi_c[:],
                in1=lnbr_sb[:, sl], op0=MULT, op1=ADD)
            nc.vector.scalar_tensor_tensor(
                out=y_sb[:, sl], in0=hpre_sb[:, sl], scalar=istd_c[:],
                in1=lnsr_sb[:, sl], op0=MULT, op1=MULT)
            nc.vector.tensor_add(out_sb[:, sl], y_sb[:, sl], t2_sb[:, sl])
        else:
            # ln_scale==1, ln_bias==0: out = hpre*istd - m*istd
            nc.vector.tensor_scalar(
                out=out_sb[:, sl], in0=hpre_sb[:, sl],
                scalar1=istd_c[:], scalar2=nmi_c[:], op0=MULT, op1=ADD)
        nc.sync.dma_start(out=out_d.ap()[:, sl], in_=out_sb[:, sl])


def _dr_layout(m, scale):
    """[1024 k, 1024 out] f32 -> [128, (h j i n)] fp8 DoubleRow layout."""
    a = np.asarray(m * scale, dtype=NP_F8)
    # k -> (j, i, p), out -> (h, n); final [p, h, j, i, n]
    a = a.reshape(JT, 2, 128, NH, 512).transpose(2, 3, 0, 1, 4)
    return np.ascontiguousarray(a.reshape(128, KT * D))


def _prep_in_maps(inputs, general_ln):
    def f32c(x):
        return np.ascontiguousarray(np.asarray(x, dtype=np.float32))

    h_a = f32c(inputs["h_A"])
    alpha = f32c(inputs["alpha"])
    pool = np.asarray(inputs["pool_vectors"], dtype=np.float32)
    w_base = np.asarray(inputs["W_base"], dtype=np.float32)

    # pool_vectors rows: [U_n (D*R) | V_n (R*D) | bias_n (D)]
    u = pool[:, :D * R_RANK].reshape(N_EXP, D, R_RANK)
    v = pool[:, D * R_RANK:2 * D * R_RANK].reshape(N_EXP, R_RANK, D)
    bias_pool = pool[:, 2 * D * R_RANK:]                    # [64, D]
    bb = np.asarray(inputs["b_base"], dtype=np.float32).reshape(1, D)
    # fp8 weights are scaled x32; alpha carries 1/32, so the bias rows
    # need x(32*32) for the pool part and x32 for b_base
    bp = np.concatenate([bias_pool * (WSC * WSC), bb * WSC], axis=0)
    bp = np.ascontiguousarray(np.asarray(bp, dtype=NP_BF16))
    vt = _dr_layout(v.reshape(N_EXP * R_RANK, D).T, WSC)   # [a, (n,r)]
    wt = _dr_layout(w_base.T, WSC)                          # [a, c]
    ut = _dr_layout(u.transpose(0, 2, 1).reshape(N_EXP * R_RANK, D), WSC)
    gm = f32c(inputs["gamma"]).reshape(1, 1) / WSC
    al_s = alpha / WSC

    in_maps = []
    for k in range(N_CORES):
        rows = slice(B_CORE * k, B_CORE * (k + 1))
        xt = h_a[rows].T                                    # [1024, 32]
        x8 = np.asarray(xt.reshape(JT, 2, 128, B_CORE).transpose(2, 0, 1, 3)
                        .reshape(128, JT * 2 * B_CORE), dtype=NP_F8)
        alt = np.concatenate(
            [al_s[rows], np.ones((B_CORE, 1), np.float32)], axis=1).T
        im = {
            "ha": f32c(h_a[rows]), "al": f32c(al_s[rows]),
            "x8": np.ascontiguousarray(x8),
            "alt": np.ascontiguousarray(np.asarray(alt, dtype=NP_BF16)),
            "vt": vt, "wt": wt, "ut": ut, "bp": bp, "gm": gm,
        }
        if general_ln:
            im["lns"] = f32c(inputs["ln_scale"]).reshape(1, D)
            im["lnb"] = f32c(inputs["ln_bias"]).reshape(1, D)
        in_maps.append(im)
    return in_maps


def _is_general_ln(inputs):
    lns = np.asarray(inputs["ln_scale"], dtype=np.float32)
    lnb = np.asarray(inputs["ln_bias"], dtype=np.float32)
    return not (np.all(lns == 1.0) and np.all(lnb == 0.0))


def get_compiled(general_ln=False):
    key = bool(general_ln)
    if key not in _COMPILED:
        _COMPILED[key] = _build(key)
    return _COMPILED[key]


def kernel(**inputs):
    general_ln = _is_general_ln(inputs)
    nc = get_compiled(general_ln)
    in_maps = _prep_in_maps(inputs, general_ln)
    res = bass_utils.run_bass_kernel_spmd(
        nc, in_maps, core_ids=list(range(N_CORES)))
    return np.concatenate([r["out"] for r in res.results], axis=0)


# revision 5
# speedup vs baseline: 1.6571x; 1.1106x over previous
"""Trainium2 Bass kernel for the DWA middle layer (moe_routing).

Math (factored form of the reference; W_assembled is never materialized):
    t     = h_A @ V_flat^T                      # [B, N*R]
    s     = t * repeat(alpha, R, axis=1)        # [B, N*R]
    h_T   = s @ U_flat^T + h_A @ W_base^T + [alpha, 1] @ [bias_pool; b_base]
    out   = LayerNorm(h_A + gamma * h_T) * ln_scale + ln_bias

Sharding: data-parallel over the batch dim (32 rows per core, 8 cores);
weights replicated.  The kernel is HBM-bound on the three 1024x1024
weight streams, so they are sent as fp8 e4m3 (scaled x32 on the host;
the scale is folded into alpha, the bias matrix and gamma, so the
device math is unchanged up to fp8 rounding — measured end-to-end
rel-err ~1.7e-3 against the fp32 reference, well inside the 2e-2 gate).
Matmuls run in DoubleRow fp8 mode (256-deep contraction per pass,
~512 PE cycles per [256k x 32m x 512n] instruction).

Layout/perf notes (from perfetto traces of earlier revisions):
  - A short stream of dummy fp8 matmuls at kernel start keeps the PE
    HAM activity window busy so the real matmuls run at 2.4 GHz
    instead of the 1.2 GHz cold clock.
  - The framework's end-of-kernel semaphore drain costs ~100ns per
    semaphore per engine, so cross-engine deps are minimized: small
    inputs ride in two concatenated blob DMAs, weights move as six
    512KB chunks, and the eight s^T transposes land in two PSUM banks
    copied out by two DVE ops.
  - Host-side prep only re-lays-out / scales / casts inputs; all
    arithmetic between tensors runs on device.
"""

import os
from contextlib import ExitStack

import ml_dtypes
import numpy as np

import concourse.bacc as bacc
import concourse.mybir as mybir
import concourse.tile as tile
from concourse import bass_utils, masks

F32 = mybir.dt.float32
BF16 = mybir.dt.bfloat16
F8 = mybir.dt.float8e4
NP_F8 = ml_dtypes.float8_e4m3
NP_BF16 = ml_dtypes.bfloat16

D = 1024          # d_A == d_B
B_CORE = 32       # batch rows per core
N_EXP = 64        # experts
R_RANK = 16       # rank per expert
N_CORES = 8
KT = 8            # 128-deep contraction tiles
JT = 4            # DoubleRow 256-deep contraction tiles
NH = 2            # output halves of 512
WSC = 32.0        # fp8 weight scale (folded into alpha/bias/gamma)
N_WU = int(os.environ.get("DWA_WARMUP_MM", "14"))  # PE warm-up matmuls

_COMPILED = {}


def _build(general_ln):
    nc = bacc.Bacc("TRN2", debug=False, num_devices=N_CORES,
                   enable_partition_id=False)

    # f32 blob: [32, 1024 (h_A) | 64 (alpha/32) | 1 (gamma/32)]
    fb_d = nc.dram_tensor("fb", [B_CORE, D + N_EXP + 1], F32,
                          kind="ExternalInput")
    # bf16 blob: [65, 1024 (bias rows) | 32 (alphaT/32 with ones row)]
    bb_d = nc.dram_tensor("bb", [N_EXP + 1, D + B_CORE], BF16,
                          kind="ExternalInput")
    x8_d = nc.dram_tensor("x8", [128, JT * 2 * B_CORE], F8,
                          kind="ExternalInput")
    # weights in DoubleRow-ready layout [128, (h j i n)] (see _dr_layout)
    vt_d = nc.dram_tensor("vt", [128, KT * D], F8, kind="ExternalInput")
    wt_d = nc.dram_tensor("wt", [128, KT * D], F8, kind="ExternalInput")
    ut_d = nc.dram_tensor("ut", [128, KT * D], F8, kind="ExternalInput")
    if general_ln:
        lns_d = nc.dram_tensor("lns", [1, D], F32, kind="ExternalInput")
        lnb_d = nc.dram_tensor("lnb", [1, D], F32, kind="ExternalInput")
    else:
        lns_d = lnb_d = None
    out_d = nc.dram_tensor("out", [B_CORE, D], F32, kind="ExternalOutput")

    with ExitStack() as ctx:
        tc = ctx.enter_context(tile.TileContext(nc))
        _emit(ctx, tc, general_ln, fb_d, bb_d, x8_d, vt_d, wt_d, ut_d,
              lns_d, lnb_d, out_d)

    nc.compile()
    return nc


def _emit(ctx, tc, general_ln, fb_d, bb_d, x8_d, vt_d, wt_d, ut_d,
          lns_d, lnb_d, out_d):
    nc = tc.nc
    MULT = mybir.AluOpType.mult
    ADD = mybir.AluOpType.add
    SQRT = mybir.ActivationFunctionType.Sqrt
    DR = mybir.MatmulPerfMode.DoubleRow

    wpool = ctx.enter_context(tc.tile_pool(name="weights", bufs=1))
    sm = ctx.enter_context(tc.tile_pool(name="small", bufs=1))
    pp = ctx.enter_context(tc.tile_pool(name="psum", bufs=1, space="PSUM"))

    vt_sb = wpool.tile([128, KT * D], F8, tag="vt")
    wt_sb = wpool.tile([128, KT * D], F8, tag="wt")
    ut_sb = wpool.tile([128, KT * D], F8, tag="ut")

    fb_sb = sm.tile([B_CORE, D + N_EXP + 1], F32, tag="fb")
    ha_sb = fb_sb[:, :D]
    al_sb = fb_sb[:, D:D + N_EXP]
    gmc_sb = fb_sb[:, D + N_EXP:D + N_EXP + 1]
    bb_sb = sm.tile([N_EXP + 1, D + B_CORE], BF16, tag="bb")
    bp_sb = bb_sb[:, :D]
    alt_sb = bb_sb[:, D:D + B_CORE]
    x8_sb = sm.tile([128, JT * 2 * B_CORE], F8, tag="x8")
    wu_sb = sm.tile([128, 2 * B_CORE + 256], F8, tag="wu")
    ident = sm.tile([B_CORE, B_CORE], F32, tag="ident")
    s_sb = sm.tile([B_CORE, D], F32, tag="s")
    st_sb = sm.tile([128, KT * B_CORE], F8, tag="st")    # s^T tiles
    hpre_sb = sm.tile([B_CORE, D], F32, tag="hpre")
    sq_sb = sm.tile([B_CORE, D], F32, tag="sq")
    out_sb = sm.tile([B_CORE, D], F32, tag="out")
    sum_h = [sm.tile([B_CORE, 1], F32, tag=f"sumh{h}", name=f"sumh{h}")
             for h in range(NH)]
    ssq_h = [sm.tile([B_CORE, 1], F32, tag=f"ssqh{h}", name=f"ssqh{h}")
             for h in range(NH)]
    m_c = sm.tile([B_CORE, 1], F32, tag="mc")
    ex2_c = sm.tile([B_CORE, 1], F32, tag="ex2c")
    msq_c = sm.tile([B_CORE, 1], F32, tag="msqc")
    var_c = sm.tile([B_CORE, 1], F32, tag="varc")
    std_c = sm.tile([B_CORE, 1], F32, tag="stdc")
    istd_c = sm.tile([B_CORE, 1], F32, tag="istdc")
    nmi_c = sm.tile([B_CORE, 1], F32, tag="nmic")
    eps_c = sm.tile([B_CORE, 1], F32, tag="epsc")
    warm_c = sm.tile([B_CORE, 1], F32, tag="warmc")
    if general_ln:
        lnsr_sb = sm.tile([B_CORE, D], F32, tag="lnsr")
        lnbr_sb = sm.tile([B_CORE, D], F32, tag="lnbr")
        y_sb = sm.tile([B_CORE, D], F32, tag="y")
        t2_sb = sm.tile([B_CORE, D], F32, tag="t2")

    # ---- weight chunk stream: V -> W -> U, one 512KB h-half per DMA,
    # the two halves of each matrix split across both HWDGE rings ----
    HH = JT * D  # 4096 fp8 columns = one output half
    for w_sb, w_d in ((vt_sb, vt_d), (wt_sb, wt_d), (ut_sb, ut_d)):
        for h in range(2):
            eng = (nc.sync, nc.scalar)[h]
            csl = slice(HH * h, HH * (h + 1))
            eng.dma_start(out=w_sb[:, csl], in_=w_d.ap()[:, csl])

    # small tensors ride the parallel SWDGE ring
    nc.gpsimd.dma_start(out=x8_sb[:], in_=x8_d.ap())
    nc.gpsimd.dma_start(out=fb_sb[:], in_=fb_d.ap())
    nc.gpsimd.dma_start(out=bb_sb[:], in_=bb_d.ap())
    if general_ln:
        nc.gpsimd.dma_start(out=lnsr_sb[:],
                            in_=lns_d.ap().broadcast_to([B_CORE, D]))
        nc.gpsimd.dma_start(out=lnbr_sb[:],
                            in_=lnb_d.ap().broadcast_to([B_CORE, D]))

    nc.vector.memset(eps_c[:], 1e-5)
    nc.vector.memset(wu_sb[:], 0.25)
    masks.make_identity(nc, ident[:])
    # preload both ACT tables (Square, Sqrt) off the critical path
    nc.scalar.activation(warm_c[:], eps_c[:],
                         mybir.ActivationFunctionType.Square)
    nc.scalar.activation(warm_c[:], eps_c[:], SQRT, bias=eps_c[:], scale=1.0)

    def dr_view(ap, n):
        return ap.rearrange("p (two n) -> p two n", two=2)

    def dr_rhs(w_sb, h, j):
        off = h * HH + j * D
        return dr_view(w_sb[:, off:off + D], 512)

    def dr_lhs(x_sb, j):
        off = j * 2 * B_CORE
        return dr_view(x_sb[:, off:off + 2 * B_CORE], B_CORE)

    t_ps = [pp.tile([B_CORE, 512], F32, tag=f"t{h}", name=f"t_ps{h}")
            for h in range(NH)]
    h_ps = [pp.tile([B_CORE, 512], F32, tag=f"h{h}", name=f"h_ps{h}")
            for h in range(NH)]
    tr_ps = [pp.tile([128, 128], F32, tag=f"tr{h}", name=f"tr_ps{h}")
             for h in range(NH)]
    wu_ps = pp.tile([B_CORE, 256], F32, tag="wu", name="wu_ps")

    # ---- PE warm-up: keep the HAM activity window busy while the
    # first weight chunks stream in, so real matmuls run at 2.4 GHz ----
    wu_lhs = dr_view(wu_sb[:, :2 * B_CORE], B_CORE)
    wu_rhs = dr_view(wu_sb[:, 2 * B_CORE:], 128)
    for i in range(N_WU):
        nc.tensor.matmul(wu_ps[:, :128], wu_lhs, wu_rhs,
                         start=True, stop=True, perf_mode=DR)

    # ---- t = h_A @ V^T ; s = t * repeat(alpha/32, R); s^T tiles ----
    for h in range(NH):
        for j in range(JT):
            nc.tensor.matmul(t_ps[h][:], dr_lhs(x8_sb, j), dr_rhs(vt_sb, h, j),
                             start=(j == 0), stop=(j == JT - 1), perf_mode=DR)
        o3 = s_sb[:, 512 * h:512 * (h + 1)].rearrange(
            "p (n r) -> p n r", r=R_RANK)
        i3 = t_ps[h][:].rearrange("p (n r) -> p n r", r=R_RANK)
        a3 = al_sb[:, 32 * h:32 * (h + 1)].unsqueeze(-1).broadcast_to(
            [B_CORE, 32, R_RANK])
        nc.vector.tensor_mul(o3, i3, a3)
        # four transposes into one PSUM bank, one fp8-casting copy out
        for kk in range(4):
            k = 4 * h + kk
            nc.tensor.transpose(tr_ps[h][:, 32 * kk:32 * (kk + 1)],
                                s_sb[:, 128 * k:128 * (k + 1)], ident[:])
        nc.vector.tensor_copy(st_sb[:, 128 * h:128 * (h + 1)], tr_ps[h][:])

    # ---- 32*h_T = [al/32,1]@bp' + h_A @ (32W)^T + s @ (32U)^T ----
    for h in range(NH):
        nc.tensor.matmul(h_ps[h][:], alt_sb[:],
                         bp_sb[:, 512 * h:512 * (h + 1)],
                         start=True, stop=False)
    for h in range(NH):
        for j in range(JT):
            nc.tensor.matmul(h_ps[h][:], dr_lhs(x8_sb, j), dr_rhs(wt_sb, h, j),
                             start=False, stop=False, perf_mode=DR)
    for h in range(NH):
        for j in range(JT):
            nc.tensor.matmul(h_ps[h][:], dr_lhs(st_sb, j), dr_rhs(ut_sb, h, j),
                             start=False, stop=(j == JT - 1), perf_mode=DR)
        sl = slice(512 * h, 512 * (h + 1))
        # h_pre = (gamma/32) * (32 h_T) + h_A, with row-sums for the mean
        nc.vector.scalar_tensor_tensor(
            out=hpre_sb[:, sl], in0=h_ps[h][:], scalar=gmc_sb,
            in1=ha_sb[:, sl], op0=MULT, op1=ADD,
            accum_out=sum_h[h][:])
        # row-sums of squares on the Scalar engine (parallel to DVE)
        nc.scalar.activation(sq_sb[:, sl], hpre_sb[:, sl],
                             mybir.ActivationFunctionType.Square,
                             accum_out=ssq_h[h][:])

    # ---- LayerNorm via E[x^2] - E[x]^2 ----
    nc.vector.tensor_scalar(out=m_c[:], in0=sum_h[0][:], scalar1=sum_h[1][:],
                            scalar2=1.0 / D, op0=ADD, op1=MULT)
    nc.vector.tensor_scalar(out=ex2_c[:], in0=ssq_h[0][:], scalar1=ssq_h[1][:],
                            scalar2=1.0 / D, op0=ADD, op1=MULT)
    nc.vector.tensor_mul(msq_c[:], m_c[:], m_c[:])
    nc.vector.tensor_sub(var_c[:], ex2_c[:], msq_c[:])
    nc.scalar.activation(std_c[:], var_c[:], SQRT, bias=eps_c[:], scale=1.0)
    nc.vector.reciprocal(istd_c[:], std_c[:])
    nc.vector.tensor_scalar(out=nmi_c[:], in0=m_c[:], scalar1=istd_c[:],
                            scalar2=-1.0, op0=MULT, op1=MULT)

    for h in range(NH):
        sl = slice(512 * h, 512 * (h + 1))
        if general_ln:
            # out = hpre*istd*lns + (lnb - m*istd*lns)
            nc.vector.scalar_tensor_tensor(
                out=t2_sb[:, sl], in0=lnsr_sb[:, sl], scalar=nmi_c[:],
                in1=lnbr_sb[:, sl], op0=MULT, op1=ADD)
            nc.vector.scalar_tensor_tensor(
                out=y_sb[:, sl], in0=hpre_sb[:, sl], scalar=istd_c[:],
                in1=lnsr_sb[:, sl], op0=MULT, op1=MULT)
            nc.vector.tensor_add(out_sb[:, sl], y_sb[:, sl], t2_sb[:, sl])
        else:
            # ln_scale==1, ln_bias==0: out = hpre*istd - m*istd
            nc.vector.tensor_scalar(
                out=out_sb[:, sl], in0=hpre_sb[:, sl],
                scalar1=istd_c[:], scalar2=nmi_c[:], op0=MULT, op1=ADD)
        nc.sync.dma_start(out=out_d.ap()[:, sl], in_=out_sb[:, sl])


def _dr_layout(m, scale):
    """[1024 k, 1024 out] f32 -> [128, (h j i n)] fp8 DoubleRow layout."""
    a = np.asarray(m * scale, dtype=NP_F8)
    # k -> (j, i, p), out -> (h, n); final [p, h, j, i, n]
    a = a.reshape(JT, 2, 128, NH, 512).transpose(2, 3, 0, 1, 4)
    return np.ascontiguousarray(a.reshape(128, KT * D))


def _prep_in_maps(inputs, general_ln):
    def f32c(x):
        return np.ascontiguousarray(np.asarray(x, dtype=np.float32))

    h_a = f32c(inputs["h_A"])
    alpha = f32c(inputs["alpha"])
    pool = np.asarray(inputs["pool_vectors"], dtype=np.float32)
    w_base = np.asarray(inputs["W_base"], dtype=np.float32)

    # pool_vectors rows: [U_n (D*R) | V_n (R*D) | bias_n (D)]
    u = pool[:, :D * R_RANK].reshape(N_EXP, D, R_RANK)
    v = pool[:, D * R_RANK:2 * D * R_RANK].reshape(N_EXP, R_RANK, D)
    bias_pool = pool[:, 2 * D * R_RANK:]                    # [64, D]
    bb = np.asarray(inputs["b_base"], dtype=np.float32).reshape(1, D)
    # fp8 weights are scaled x32; alpha carries 1/32, so the bias rows
    # need x(32*32) for the pool part and x32 for b_base
    bp = np.concatenate([bias_pool * (WSC * WSC), bb * WSC], axis=0)
    vt = _dr_layout(v.reshape(N_EXP * R_RANK, D).T, WSC)   # [a, (n,r)]
    wt = _dr_layout(w_base.T, WSC)                          # [a, c]
    ut = _dr_layout(u.transpose(0, 2, 1).reshape(N_EXP * R_RANK, D), WSC)
    gm = float(np.asarray(inputs["gamma"], dtype=np.float32)) / WSC
    al_s = alpha / WSC

    in_maps = []
    for k in range(N_CORES):
        rows = slice(B_CORE * k, B_CORE * (k + 1))
        xt = h_a[rows].T                                    # [1024, 32]
        x8 = np.asarray(xt.reshape(JT, 2, 128, B_CORE).transpose(2, 0, 1, 3)
                        .reshape(128, JT * 2 * B_CORE), dtype=NP_F8)
        fb = np.concatenate(
            [h_a[rows], al_s[rows],
             np.full((B_CORE, 1), gm, np.float32)], axis=1)
        alt = np.concatenate(
            [al_s[rows], np.ones((B_CORE, 1), np.float32)], axis=1).T
        bbb = np.concatenate([bp, alt], axis=1)             # [65, 1056]
        im = {
            "fb": f32c(fb),
            "bb": np.ascontiguousarray(np.asarray(bbb, dtype=NP_BF16)),
            "x8": np.ascontiguousarray(x8),
            "vt": vt, "wt": wt, "ut": ut,
        }
        if general_ln:
            im["lns"] = f32c(inputs["ln_scale"]).reshape(1, D)
            im["lnb"] = f32c(inputs["ln_bias"]).reshape(1, D)
        in_maps.append(im)
    return in_maps


def _is_general_ln(inputs):
    lns = np.asarray(inputs["ln_scale"], dtype=np.float32)
    lnb = np.asarray(inputs["ln_bias"], dtype=np.float32)
    return not (np.all(lns == 1.0) and np.all(lnb == 0.0))


def get_compiled(general_ln=False):
    key = bool(general_ln)
    if key not in _COMPILED:
        _COMPILED[key] = _build(key)
    return _COMPILED[key]


def kernel(**inputs):
    general_ln = _is_general_ln(inputs)
    nc = get_compiled(general_ln)
    in_maps = _prep_in_maps(inputs, general_ln)
    res = bass_utils.run_bass_kernel_spmd(
        nc, in_maps, core_ids=list(range(N_CORES)))
    return np.concatenate([r["out"] for r in res.results], axis=0)


# revision 8
# speedup vs baseline: 1.7800x; 1.0742x over previous
"""Trainium2 Bass kernel for the DWA middle layer (moe_routing).

Math (factored form of the reference; W_assembled is never materialized):
    t     = h_A @ V_flat^T                      # [B, N*R]
    s     = t * repeat(alpha, R, axis=1)        # [B, N*R]
    h_T   = s @ U_flat^T + h_A @ W_base^T + [alpha, 1] @ [bias_pool; b_base]
    out   = LayerNorm(h_A + gamma * h_T) * ln_scale + ln_bias

Sharding: data-parallel over the batch dim (32 rows per core, 8 cores);
weights replicated.  The kernel is HBM-bound on the three 1024x1024
weight streams, so they are sent as fp8 e4m3 (scaled x32 on the host;
the scale is folded into alpha, the bias matrix and gamma, so the
device math is unchanged up to fp8 rounding — measured end-to-end
rel-err ~2e-3 against the fp32 reference, well inside the 2e-2 gate).
Matmuls run in DoubleRow fp8 mode (256-deep contraction per pass,
~512 PE cycles per [256k x 32m x 512n] instruction).

Perf notes (from perfetto/NTFF analysis of earlier revisions):
  - A stream of dummy fp8 matmuls at kernel start keeps the PE HAM
    activity window busy so real matmuls run at 2.4 GHz, not the
    1.2 GHz cold clock.
  - The NEFF exit protocol drains every allocated DMA queue ring
    (~115ns x 16 rings per issuing engine), so ALL loads ride one
    HWDGE ring (sync) — the SDMA engines already round-robin between
    queues, so a second ring adds no aggregate bandwidth, only tail.
  - Per-DMA fixed overhead is ~1.3us, so small inputs are packed into
    few blob DMAs and h_A^T(fp8) is concatenated with the V matrix.
  - Host-side prep only re-lays-out / scales / casts inputs; all
    arithmetic between tensors runs on device.
"""

import os
from contextlib import ExitStack

import ml_dtypes
import numpy as np

import concourse.bacc as bacc
import concourse.mybir as mybir
import concourse.tile as tile
from concourse import bass_utils, masks

F32 = mybir.dt.float32
BF16 = mybir.dt.bfloat16
F8 = mybir.dt.float8e4
NP_F8 = ml_dtypes.float8_e4m3
NP_BF16 = ml_dtypes.bfloat16

D = 1024          # d_A == d_B
B_CORE = 32       # batch rows per core
N_EXP = 64        # experts
R_RANK = 16       # rank per expert
N_CORES = 8
KT = 8            # 128-deep contraction tiles
JT = 4            # DoubleRow 256-deep contraction tiles
NH = 2            # output halves of 512
WSC = 32.0        # fp8 weight scale (folded into alpha/bias/gamma)
XW = JT * 2 * B_CORE  # 256 columns of h_A^T tiles
N_WU = int(os.environ.get("DWA_WARMUP_MM", "26"))  # PE warm-up matmuls

_COMPILED = {}


def _build(general_ln):
    nc = bacc.Bacc("TRN2", debug=False, num_devices=N_CORES,
                   enable_partition_id=False)

    # [128, 256 (h_A^T fp8 tiles) | 8192 (V, DoubleRow layout)]
    xv_d = nc.dram_tensor("xv", [128, XW + KT * D], F8, kind="ExternalInput")
    wt_d = nc.dram_tensor("wt", [128, KT * D], F8, kind="ExternalInput")
    ut_d = nc.dram_tensor("ut", [128, KT * D], F8, kind="ExternalInput")
    # bf16 blob: [32, 1024 (h_A) | 64 (alpha/32) | 1 (gamma/32)]
    fb_d = nc.dram_tensor("fb", [B_CORE, D + N_EXP + 1], F32,
                          kind="ExternalInput")
    # bf16 blob: [65, 1024 (bias rows) | 32 (alphaT/32 with ones row)]
    bb_d = nc.dram_tensor("bb", [N_EXP + 1, D + B_CORE], BF16,
                          kind="ExternalInput")
    if general_ln:
        lns_d = nc.dram_tensor("lns", [1, D], F32, kind="ExternalInput")
        lnb_d = nc.dram_tensor("lnb", [1, D], F32, kind="ExternalInput")
    else:
        lns_d = lnb_d = None
    out_d = nc.dram_tensor("out", [B_CORE, D], F32, kind="ExternalOutput")

    with ExitStack() as ctx:
        tc = ctx.enter_context(tile.TileContext(nc))
        _emit(ctx, tc, general_ln, xv_d, wt_d, ut_d, fb_d, bb_d,
              lns_d, lnb_d, out_d)

    nc.compile()
    return nc


def _emit(ctx, tc, general_ln, xv_d, wt_d, ut_d, fb_d, bb_d,
          lns_d, lnb_d, out_d):
    nc = tc.nc
    MULT = mybir.AluOpType.mult
    ADD = mybir.AluOpType.add
    SUB = mybir.AluOpType.subtract
    SQRT = mybir.ActivationFunctionType.Sqrt
    DR = mybir.MatmulPerfMode.DoubleRow

    wpool = ctx.enter_context(tc.tile_pool(name="weights", bufs=1))
    sm = ctx.enter_context(tc.tile_pool(name="small", bufs=1))
    pp = ctx.enter_context(tc.tile_pool(name="psum", bufs=1, space="PSUM"))

    xv_sb = wpool.tile([128, XW + KT * D], F8, tag="xv")
    x8_sb = xv_sb[:, :XW]
    vt_sb = xv_sb[:, XW:]
    wt_sb = wpool.tile([128, KT * D], F8, tag="wt")
    ut_sb = wpool.tile([128, KT * D], F8, tag="ut")

    fb_sb = sm.tile([B_CORE, D + N_EXP + 1], F32, tag="fb")
    ha_sb = fb_sb[:, :D]
    al_sb = fb_sb[:, D:D + N_EXP]
    gmc_sb = fb_sb[:, D + N_EXP:D + N_EXP + 1]
    bb_sb = sm.tile([N_EXP + 1, D + B_CORE], BF16, tag="bb")
    bp_sb = bb_sb[:, :D]
    alt_sb = bb_sb[:, D:D + B_CORE]
    wu_sb = sm.tile([128, 2 * B_CORE + 256], F8, tag="wu")
    ident = sm.tile([B_CORE, B_CORE], F32, tag="ident")
    s_sb = sm.tile([B_CORE, D], F32, tag="s")
    st_sb = sm.tile([128, KT * B_CORE], F8, tag="st")    # s^T tiles
    hpre_sb = sm.tile([B_CORE, D], F32, tag="hpre")
    sq_sb = sm.tile([B_CORE, D], F32, tag="sq")
    out_sb = sm.tile([B_CORE, D], F32, tag="out")
    sum_h = [sm.tile([B_CORE, 1], F32, tag=f"sumh{h}", name=f"sumh{h}")
             for h in range(NH)]
    ssq_a = sm.tile([B_CORE, 1], F32, tag="ssqa")
    ssq_b = sm.tile([B_CORE, 1], F32, tag="ssqb")
    ssq_c2 = sm.tile([B_CORE, 1], F32, tag="ssqc2")
    m_c = sm.tile([B_CORE, 1], F32, tag="mc")
    ssqs_c = sm.tile([B_CORE, 1], F32, tag="ssqsc")
    msq_c = sm.tile([B_CORE, 1], F32, tag="msqc")
    var_c = sm.tile([B_CORE, 1], F32, tag="varc")
    std_c = sm.tile([B_CORE, 1], F32, tag="stdc")
    istd_c = sm.tile([B_CORE, 1], F32, tag="istdc")
    nmi_c = sm.tile([B_CORE, 1], F32, tag="nmic")
    eps_c = sm.tile([B_CORE, 1], F32, tag="epsc")
    warm_c = sm.tile([B_CORE, 1], F32, tag="warmc")
    if general_ln:
        lnsr_sb = sm.tile([B_CORE, D], F32, tag="lnsr")
        lnbr_sb = sm.tile([B_CORE, D], F32, tag="lnbr")
        y_sb = sm.tile([B_CORE, D], F32, tag="y")
        t2_sb = sm.tile([B_CORE, D], F32, tag="t2")

    # ---- all loads on ONE HWDGE ring, in consumption order; the last
    # weight matrix (U) gates the final matmuls of each accumulation ----
    nc.sync.dma_start(out=xv_sb[:], in_=xv_d.ap())
    nc.sync.dma_start(out=fb_sb[:], in_=fb_d.ap())
    nc.sync.dma_start(out=bb_sb[:], in_=bb_d.ap())
    nc.sync.dma_start(out=wt_sb[:], in_=wt_d.ap())
    nc.sync.dma_start(out=ut_sb[:], in_=ut_d.ap())
    if general_ln:
        nc.sync.dma_start(out=lnsr_sb[:],
                          in_=lns_d.ap().broadcast_to([B_CORE, D]))
        nc.sync.dma_start(out=lnbr_sb[:],
                          in_=lnb_d.ap().broadcast_to([B_CORE, D]))

    nc.vector.memset(eps_c[:], 1e-5)
    nc.vector.memset(wu_sb[:], 0.25)
    masks.make_identity(nc, ident[:])
    # preload both ACT tables (Square, Sqrt) off the critical path
    nc.scalar.activation(warm_c[:], eps_c[:],
                         mybir.ActivationFunctionType.Square)
    nc.scalar.activation(warm_c[:], eps_c[:], SQRT, bias=eps_c[:], scale=1.0)

    def dr_view(ap):
        return ap.rearrange("p (two n) -> p two n", two=2)

    def dr_rhs(w_sb, h, j):
        off = h * (JT * D) + j * D
        return dr_view(w_sb[:, off:off + D])

    def dr_lhs(x_sb, j):
        off = j * 2 * B_CORE
        return dr_view(x_sb[:, off:off + 2 * B_CORE])

    t_ps = [pp.tile([B_CORE, 512], F32, tag=f"t{h}", name=f"t_ps{h}")
            for h in range(NH)]
    h_ps = [pp.tile([B_CORE, 512], F32, tag=f"h{h}", name=f"h_ps{h}")
            for h in range(NH)]
    tr_ps = [pp.tile([128, 128], F32, tag=f"tr{h}", name=f"tr_ps{h}")
             for h in range(NH)]
    wu_ps = pp.tile([B_CORE, 128], F32, tag="wu", name="wu_ps")

    # ---- PE warm-up: keep the HAM activity window busy while the
    # first weight chunks stream in, so real matmuls run at 2.4 GHz ----
    wu_lhs = dr_view(wu_sb[:, :2 * B_CORE])
    wu_rhs = dr_view(wu_sb[:, 2 * B_CORE:])
    for i in range(N_WU):
        nc.tensor.matmul(wu_ps[:], wu_lhs, wu_rhs,
                         start=True, stop=True, perf_mode=DR)

    # ---- t = h_A @ V^T ; s = t * repeat(alpha/32, R); s^T tiles ----
    for h in range(NH):
        for j in range(JT):
            nc.tensor.matmul(t_ps[h][:], dr_lhs(x8_sb, j), dr_rhs(vt_sb, h, j),
                             start=(j == 0), stop=(j == JT - 1), perf_mode=DR)
        o3 = s_sb[:, 512 * h:512 * (h + 1)].rearrange(
            "p (n r) -> p n r", r=R_RANK)
        i3 = t_ps[h][:].rearrange("p (n r) -> p n r", r=R_RANK)
        a3 = al_sb[:, 32 * h:32 * (h + 1)].unsqueeze(-1).broadcast_to(
            [B_CORE, 32, R_RANK])
        nc.vector.tensor_mul(o3, i3, a3)
        # four transposes into one PSUM bank, one fp8-casting copy out
        for kk in range(4):
            k = 4 * h + kk
            nc.tensor.transpose(tr_ps[h][:, 32 * kk:32 * (kk + 1)],
                                s_sb[:, 128 * k:128 * (k + 1)], ident[:])
        nc.vector.tensor_copy(st_sb[:, 128 * h:128 * (h + 1)], tr_ps[h][:])

    # ---- 32*h_T = [al/32,1]@bp' + h_A @ (32W)^T + s @ (32U)^T ----
    # U arrives last, so U matmuls close each accumulation group.
    for h in range(NH):
        nc.tensor.matmul(h_ps[h][:], alt_sb[:],
                         bp_sb[:, 512 * h:512 * (h + 1)],
                         start=True, stop=False)
    for h in range(NH):
        for j in range(JT):
            nc.tensor.matmul(h_ps[h][:], dr_lhs(x8_sb, j), dr_rhs(wt_sb, h, j),
                             start=False, stop=False, perf_mode=DR)
    for h in range(NH):
        for j in range(JT):
            nc.tensor.matmul(h_ps[h][:], dr_lhs(st_sb, j), dr_rhs(ut_sb, h, j),
                             start=False, stop=(j == JT - 1), perf_mode=DR)
        sl = slice(512 * h, 512 * (h + 1))
        # h_pre = (gamma/32) * (32 h_T) + h_A, with row-sums for the mean
        nc.vector.scalar_tensor_tensor(
            out=hpre_sb[:, sl], in0=h_ps[h][:], scalar=gmc_sb,
            in1=ha_sb[:, sl], op0=MULT, op1=ADD,
            accum_out=sum_h[h][:])
        # row-sums of squares on the Scalar engine (parallel to DVE)
        nc.scalar.activation(sq_sb[:, sl], hpre_sb[:, sl],
                             mybir.ActivationFunctionType.Square,
                             accum_out=(ssq_a if h == 0 else ssq_b)[:])

    # ---- LayerNorm via E[x^2] - E[x]^2 ----
    nc.vector.tensor_scalar(out=m_c[:], in0=sum_h[0][:], scalar1=sum_h[1][:],
                            scalar2=1.0 / D, op0=ADD, op1=MULT)
    nc.vector.tensor_add(ssqs_c[:], ssq_a[:], ssq_b[:])
    nc.vector.tensor_mul(msq_c[:], m_c[:], m_c[:])
    nc.vector.scalar_tensor_tensor(
        out=var_c[:], in0=ssqs_c[:], scalar=1.0 / D, in1=msq_c[:],
        op0=MULT, op1=SUB)
    nc.scalar.activation(std_c[:], var_c[:], SQRT, bias=eps_c[:], scale=1.0)
    nc.vector.reciprocal(istd_c[:], std_c[:])
    nc.vector.tensor_scalar(out=nmi_c[:], in0=m_c[:], scalar1=istd_c[:],
                            scalar2=-1.0, op0=MULT, op1=MULT)

    for h in range(NH):
        sl = slice(512 * h, 512 * (h + 1))
        if general_ln:
            # out = hpre*istd*lns + (lnb - m*istd*lns)
            nc.vector.scalar_tensor_tensor(
                out=t2_sb[:, sl], in0=lnsr_sb[:, sl], scalar=nmi_c[:],
                in1=lnbr_sb[:, sl], op0=MULT, op1=ADD)
            nc.vector.scalar_tensor_tensor(
                out=y_sb[:, sl], in0=hpre_sb[:, sl], scalar=istd_c[:],
                in1=lnsr_sb[:, sl], op0=MULT, op1=MULT)
            nc.vector.tensor_add(out_sb[:, sl], y_sb[:, sl], t2_sb[:, sl])
        else:
            # ln_scale==1, ln_bias==0: out = hpre*istd - m*istd
            nc.vector.tensor_scalar(
                out=out_sb[:, sl], in0=hpre_sb[:, sl],
                scalar1=istd_c[:], scalar2=nmi_c[:], op0=MULT, op1=ADD)
        nc.sync.dma_start(out=out_d.ap()[:, sl], in_=out_sb[:, sl])


def _dr_layout(m, scale):
    """[1024 k, 1024 out] f32 -> [128, (h j i n)] fp8 DoubleRow layout."""
    a = np.asarray(m * scale, dtype=NP_F8)
    # k -> (j, i, p), out -> (h, n); final [p, h, j, i, n]
    a = a.reshape(JT, 2, 128, NH, 512).transpose(2, 3, 0, 1, 4)
    return np.ascontiguousarray(a.reshape(128, KT * D))


def _prep_in_maps(inputs, general_ln):
    def f32c(x):
        return np.ascontiguousarray(np.asarray(x, dtype=np.float32))

    h_a = f32c(inputs["h_A"])
    alpha = f32c(inputs["alpha"])
    pool = np.asarray(inputs["pool_vectors"], dtype=np.float32)
    w_base = np.asarray(inputs["W_base"], dtype=np.float32)

    # pool_vectors rows: [U_n (D*R) | V_n (R*D) | bias_n (D)]
    u = pool[:, :D * R_RANK].reshape(N_EXP, D, R_RANK)
    v = pool[:, D * R_RANK:2 * D * R_RANK].reshape(N_EXP, R_RANK, D)
    bias_pool = pool[:, 2 * D * R_RANK:]                    # [64, D]
    bb = np.asarray(inputs["b_base"], dtype=np.float32).reshape(1, D)
    # fp8 weights are scaled x32; alpha carries 1/32, so the bias rows
    # need x(32*32) for the pool part and x32 for b_base
    bp = np.concatenate([bias_pool * (WSC * WSC), bb * WSC], axis=0)
    vt = _dr_layout(v.reshape(N_EXP * R_RANK, D).T, WSC)   # [a, (n,r)]
    wt = _dr_layout(w_base.T, WSC)                          # [a, c]
    ut = _dr_layout(u.transpose(0, 2, 1).reshape(N_EXP * R_RANK, D), WSC)
    gm = float(np.asarray(inputs["gamma"], dtype=np.float32)) / WSC
    al_s = alpha / WSC

    in_maps = []
    for k in range(N_CORES):
        rows = slice(B_CORE * k, B_CORE * (k + 1))
        xt = h_a[rows].T                                    # [1024, 32]
        x8 = np.asarray(xt.reshape(JT, 2, 128, B_CORE).transpose(2, 0, 1, 3)
                        .reshape(128, XW), dtype=NP_F8)
        xv = np.concatenate([x8, vt], axis=1)               # [128, 8448]
        fb = np.concatenate(
            [h_a[rows], al_s[rows],
             np.full((B_CORE, 1), gm, np.float32)], axis=1)
        alt = np.concatenate(
            [al_s[rows], np.ones((B_CORE, 1), np.float32)], axis=1).T
        bbb = np.concatenate([bp, alt], axis=1)             # [65, 1056]
        im = {
            "xv": np.ascontiguousarray(xv),
            "fb": f32c(fb),
            "bb": np.ascontiguousarray(np.asarray(bbb, dtype=NP_BF16)),
            "wt": wt, "ut": ut,
        }
        if general_ln:
            im["lns"] = f32c(inputs["ln_scale"]).reshape(1, D)
            im["lnb"] = f32c(inputs["ln_bias"]).reshape(1, D)
        in_maps.append(im)
    return in_maps


def _is_general_ln(inputs):
    lns = np.asarray(inputs["ln_scale"], dtype=np.float32)
    lnb = np.asarray(inputs["ln_bias"], dtype=np.float32)
    return not (np.all(lns == 1.0) and np.all(lnb == 0.0))


def get_compiled(general_ln=False):
    key = bool(general_ln)
    if key not in _COMPILED:
        _COMPILED[key] = _build(key)
    return _COMPILED[key]


def kernel(**inputs):
    general_ln = _is_general_ln(inputs)
    nc = get_compiled(general_ln)
    in_maps = _prep_in_maps(inputs, general_ln)
    res = bass_utils.run_bass_kernel_spmd(
        nc, in_maps, core_ids=list(range(N_CORES)))
    return np.concatenate([r["out"] for r in res.results], axis=0)


# revision 10
# speedup vs baseline: 1.8124x; 1.0182x over previous
"""Trainium2 Bass kernel for the DWA middle layer (moe_routing).

Math (factored form of the reference; W_assembled is never materialized):
    t     = h_A @ V_flat^T                      # [B, N*R]
    s     = t * repeat(alpha, R, axis=1)        # [B, N*R]
    h_T   = s @ U_flat^T + h_A @ W_base^T + [alpha, 1] @ [bias_pool; b_base]
    out   = LayerNorm(h_A + gamma * h_T) * ln_scale + ln_bias

Sharding: data-parallel over the batch dim (32 rows per core, 8 cores);
weights replicated.  The kernel is HBM-bound on the three 1024x1024
weight streams, so they are sent as fp8 e4m3 (scaled x32 on the host;
the scale is folded into alpha, the bias matrix and gamma, so the
device math is unchanged up to fp8 rounding — measured end-to-end
rel-err ~2e-3 against the fp32 reference, well inside the 2e-2 gate).
Matmuls run in DoubleRow fp8 mode (256-deep contraction per pass,
~512 PE cycles per [256k x 32m x 512n] instruction).

Perf notes (from perfetto/NTFF analysis of earlier revisions):
  - A stream of dummy fp8 matmuls at kernel start keeps the PE HAM
    activity window busy so real matmuls run at 2.4 GHz, not the
    1.2 GHz cold clock.
  - The NEFF exit protocol drains every allocated DMA queue ring
    (~115ns x 16 rings per issuing engine), so ALL loads ride one
    HWDGE ring (sync) — the SDMA engines already round-robin between
    queues, so a second ring adds no aggregate bandwidth, only tail.
  - Per-DMA fixed overhead is ~1.3us, so small inputs are packed into
    few blob DMAs and h_A^T(fp8) is concatenated with the V matrix.
  - Host-side prep only re-lays-out / scales / casts inputs; all
    arithmetic between tensors runs on device.
"""

import os
from contextlib import ExitStack

import ml_dtypes
import numpy as np

import concourse.bacc as bacc
import concourse.mybir as mybir
import concourse.tile as tile
from concourse import bass_utils, masks

F32 = mybir.dt.float32
BF16 = mybir.dt.bfloat16
F8 = mybir.dt.float8e4
NP_F8 = ml_dtypes.float8_e4m3
NP_BF16 = ml_dtypes.bfloat16

D = 1024          # d_A == d_B
B_CORE = 32       # batch rows per core
N_EXP = 64        # experts
R_RANK = 16       # rank per expert
N_CORES = 8
KT = 8            # 128-deep contraction tiles
JT = 4            # DoubleRow 256-deep contraction tiles
NH = 2            # output halves of 512
WSC = 32.0        # fp8 weight scale (folded into alpha/bias/gamma)
XW = JT * 2 * B_CORE  # 256 columns of h_A^T tiles
N_WU = int(os.environ.get("DWA_WARMUP_MM", "12"))  # PE warm-up matmuls

_COMPILED = {}


def _build(general_ln):
    nc = bacc.Bacc("TRN2", debug=False, num_devices=N_CORES,
                   enable_partition_id=False)

    # [128, 256 (h_A^T fp8 tiles) | 8192 (V, DoubleRow layout)]
    xv_d = nc.dram_tensor("xv", [128, XW + KT * D], F8, kind="ExternalInput")
    wt_d = nc.dram_tensor("wt", [128, KT * D], F8, kind="ExternalInput")
    ut_d = nc.dram_tensor("ut", [128, KT * D], F8, kind="ExternalInput")
    # bf16 blob: [32, 1024 (h_A) | 64 (alpha/32) | 1 (gamma/32)]
    fb_d = nc.dram_tensor("fb", [B_CORE, D + N_EXP + 1], F32,
                          kind="ExternalInput")
    # bf16 blob: [65, 1024 (bias rows) | 32 (alphaT/32 with ones row)]
    bb_d = nc.dram_tensor("bb", [N_EXP + 1, D + B_CORE], BF16,
                          kind="ExternalInput")
    if general_ln:
        lns_d = nc.dram_tensor("lns", [1, D], F32, kind="ExternalInput")
        lnb_d = nc.dram_tensor("lnb", [1, D], F32, kind="ExternalInput")
    else:
        lns_d = lnb_d = None
    out_d = nc.dram_tensor("out", [B_CORE, D], F32, kind="ExternalOutput")

    with ExitStack() as ctx:
        tc = ctx.enter_context(tile.TileContext(nc))
        _emit(ctx, tc, general_ln, xv_d, wt_d, ut_d, fb_d, bb_d,
              lns_d, lnb_d, out_d)

    nc.compile()
    return nc


def _emit(ctx, tc, general_ln, xv_d, wt_d, ut_d, fb_d, bb_d,
          lns_d, lnb_d, out_d):
    nc = tc.nc
    MULT = mybir.AluOpType.mult
    ADD = mybir.AluOpType.add
    SUB = mybir.AluOpType.subtract
    SQRT = mybir.ActivationFunctionType.Sqrt
    DR = mybir.MatmulPerfMode.DoubleRow

    wpool = ctx.enter_context(tc.tile_pool(name="weights", bufs=1))
    sm = ctx.enter_context(tc.tile_pool(name="small", bufs=1))
    pp = ctx.enter_context(tc.tile_pool(name="psum", bufs=1, space="PSUM"))

    xv_sb = wpool.tile([128, XW + KT * D], F8, tag="xv")
    x8_sb = xv_sb[:, :XW]
    vt_sb = xv_sb[:, XW:]
    wt_sb = wpool.tile([128, KT * D], F8, tag="wt")
    ut_sb = wpool.tile([128, KT * D], F8, tag="ut")

    fb_sb = sm.tile([B_CORE, D + N_EXP + 1], F32, tag="fb")
    ha_sb = fb_sb[:, :D]
    al_sb = fb_sb[:, D:D + N_EXP]
    gmc_sb = fb_sb[:, D + N_EXP:D + N_EXP + 1]
    bb_sb = sm.tile([N_EXP + 1, D + B_CORE], BF16, tag="bb")
    bp_sb = bb_sb[:, :D]
    alt_sb = bb_sb[:, D:D + B_CORE]
    wu_sb = sm.tile([128, 2 * B_CORE + 256], F8, tag="wu")
    ident = sm.tile([B_CORE, B_CORE], F32, tag="ident")
    s_sb = sm.tile([B_CORE, D], F32, tag="s")
    st_sb = sm.tile([128, KT * B_CORE], F8, tag="st")    # s^T tiles
    hpre_sb = sm.tile([B_CORE, D], F32, tag="hpre")
    sq_sb = sm.tile([B_CORE, D], F32, tag="sq")
    out_sb = sm.tile([B_CORE, D], F32, tag="out")
    sum_h = [sm.tile([B_CORE, 1], F32, tag=f"sumh{h}", name=f"sumh{h}")
             for h in range(NH)]
    sum_q = sm.tile([B_CORE, 1], F32, tag="sumq")
    ssq_a = sm.tile([B_CORE, 1], F32, tag="ssqa")
    ssq_b = sm.tile([B_CORE, 1], F32, tag="ssqb")
    ssq_c2 = sm.tile([B_CORE, 1], F32, tag="ssqc2")
    m_c = sm.tile([B_CORE, 1], F32, tag="mc")
    ssqs_c = sm.tile([B_CORE, 1], F32, tag="ssqsc")
    msq_c = sm.tile([B_CORE, 1], F32, tag="msqc")
    var_c = sm.tile([B_CORE, 1], F32, tag="varc")
    std_c = sm.tile([B_CORE, 1], F32, tag="stdc")
    istd_c = sm.tile([B_CORE, 1], F32, tag="istdc")
    nmi_c = sm.tile([B_CORE, 1], F32, tag="nmic")
    eps_c = sm.tile([B_CORE, 1], F32, tag="epsc")
    warm_c = sm.tile([B_CORE, 1], F32, tag="warmc")
    if general_ln:
        lnsr_sb = sm.tile([B_CORE, D], F32, tag="lnsr")
        lnbr_sb = sm.tile([B_CORE, D], F32, tag="lnbr")
        y_sb = sm.tile([B_CORE, D], F32, tag="y")
        t2_sb = sm.tile([B_CORE, D], F32, tag="t2")

    # ---- loads split over BOTH HWDGE rings (~180 GB/s each), in strict
    # consumption order; U is chunked so its matmuls start as bytes land ----
    HH = JT * D
    nc.sync.dma_start(out=xv_sb[:, :XW + HH], in_=xv_d.ap()[:, :XW + HH])
    nc.scalar.dma_start(out=fb_sb[:], in_=fb_d.ap())
    nc.scalar.dma_start(out=xv_sb[:, XW + HH:], in_=xv_d.ap()[:, XW + HH:])
    nc.sync.dma_start(out=bb_sb[:], in_=bb_d.ap())
    nc.sync.dma_start(out=wt_sb[:, :HH], in_=wt_d.ap()[:, :HH])
    nc.scalar.dma_start(out=wt_sb[:, HH:], in_=wt_d.ap()[:, HH:])
    for q in range(4):
        eng = (nc.sync, nc.scalar)[q % 2]
        csl = slice(2048 * q, 2048 * (q + 1))
        eng.dma_start(out=ut_sb[:, csl], in_=ut_d.ap()[:, csl])
    if general_ln:
        nc.sync.dma_start(out=lnsr_sb[:],
                          in_=lns_d.ap().broadcast_to([B_CORE, D]))
        nc.scalar.dma_start(out=lnbr_sb[:],
                          in_=lnb_d.ap().broadcast_to([B_CORE, D]))

    nc.vector.memset(eps_c[:], 1e-5)
    nc.vector.memset(wu_sb[:], 0.25)
    masks.make_identity(nc, ident[:])
    # preload both ACT tables (Square, Sqrt) off the critical path
    nc.scalar.activation(warm_c[:], eps_c[:],
                         mybir.ActivationFunctionType.Square)
    nc.scalar.activation(warm_c[:], eps_c[:], SQRT, bias=eps_c[:], scale=1.0)

    def dr_view(ap):
        return ap.rearrange("p (two n) -> p two n", two=2)

    def dr_rhs(w_sb, h, j):
        off = h * (JT * D) + j * D
        return dr_view(w_sb[:, off:off + D])

    def dr_lhs(x_sb, j):
        off = j * 2 * B_CORE
        return dr_view(x_sb[:, off:off + 2 * B_CORE])

    t_ps = [pp.tile([B_CORE, 512], F32, tag=f"t{h}", name=f"t_ps{h}")
            for h in range(NH)]
    h_ps = [pp.tile([B_CORE, 512], F32, tag=f"h{h}", name=f"h_ps{h}")
            for h in range(NH)]
    tr_ps = [pp.tile([128, 128], F32, tag=f"tr{h}", name=f"tr_ps{h}")
             for h in range(NH)]
    wu_ps = pp.tile([B_CORE, 128], F32, tag="wu", name="wu_ps")

    # ---- PE warm-up: keep the HAM activity window busy while the
    # first weight chunks stream in, so real matmuls run at 2.4 GHz ----
    wu_lhs = dr_view(wu_sb[:, :2 * B_CORE])
    wu_rhs = dr_view(wu_sb[:, 2 * B_CORE:])
    for i in range(N_WU):
        nc.tensor.matmul(wu_ps[:], wu_lhs, wu_rhs,
                         start=True, stop=True, perf_mode=DR)

    # ---- t = h_A @ V^T ; s = t * repeat(alpha/32, R); s^T tiles ----
    for h in range(NH):
        for j in range(JT):
            nc.tensor.matmul(t_ps[h][:], dr_lhs(x8_sb, j), dr_rhs(vt_sb, h, j),
                             start=(j == 0), stop=(j == JT - 1), perf_mode=DR)
        o3 = s_sb[:, 512 * h:512 * (h + 1)].rearrange(
            "p (n r) -> p n r", r=R_RANK)
        i3 = t_ps[h][:].rearrange("p (n r) -> p n r", r=R_RANK)
        a3 = al_sb[:, 32 * h:32 * (h + 1)].unsqueeze(-1).broadcast_to(
            [B_CORE, 32, R_RANK])
        nc.vector.tensor_mul(o3, i3, a3)
        # four transposes into one PSUM bank, one fp8-casting copy out
        for kk in range(4):
            k = 4 * h + kk
            nc.tensor.transpose(tr_ps[h][:, 32 * kk:32 * (kk + 1)],
                                s_sb[:, 128 * k:128 * (k + 1)], ident[:])
        nc.vector.tensor_copy(st_sb[:, 128 * h:128 * (h + 1)], tr_ps[h][:])

    # ---- 32*h_T = [al/32,1]@bp' + h_A @ (32W)^T + s @ (32U)^T ----
    # U arrives last, so U matmuls close each accumulation group.
    for h in range(NH):
        nc.tensor.matmul(h_ps[h][:], alt_sb[:],
                         bp_sb[:, 512 * h:512 * (h + 1)],
                         start=True, stop=False)
    for h in range(NH):
        for j in range(JT):
            nc.tensor.matmul(h_ps[h][:], dr_lhs(x8_sb, j), dr_rhs(wt_sb, h, j),
                             start=False, stop=False, perf_mode=DR)
    for h in range(NH):
        for j in range(JT):
            nc.tensor.matmul(h_ps[h][:], dr_lhs(st_sb, j), dr_rhs(ut_sb, h, j),
                             start=False, stop=(j == JT - 1), perf_mode=DR)
        if h == 0:
            sl = slice(0, 512)
            # h_pre = (gamma/32)*(32 h_T) + h_A, with row-sums for the mean
            nc.vector.scalar_tensor_tensor(
                out=hpre_sb[:, sl], in0=h_ps[0][:], scalar=gmc_sb,
                in1=ha_sb[:, sl], op0=MULT, op1=ADD,
                accum_out=sum_h[0][:])
            nc.scalar.activation(sq_sb[:, sl], hpre_sb[:, sl],
                                 mybir.ActivationFunctionType.Square,
                                 accum_out=ssq_a[:])
        else:
            # critical-path half: quarter-split so DVE/ACT pipeline
            for qq in range(2):
                sl = slice(512 + 256 * qq, 512 + 256 * (qq + 1))
                pl = slice(256 * qq, 256 * (qq + 1))
                nc.vector.scalar_tensor_tensor(
                    out=hpre_sb[:, sl], in0=h_ps[1][:, pl], scalar=gmc_sb,
                    in1=ha_sb[:, sl], op0=MULT, op1=ADD,
                    accum_out=(sum_h[1] if qq == 0 else sum_q)[:])
                nc.scalar.activation(sq_sb[:, sl], hpre_sb[:, sl],
                                     mybir.ActivationFunctionType.Square,
                                     accum_out=(ssq_b if qq == 0 else ssq_c2)[:])

    # ---- LayerNorm via E[x^2] - E[x]^2 ----
    # m_c holds D*mean; the 1/D folds into msq and nmi scalars
    nc.vector.tensor_scalar(out=m_c[:], in0=sum_h[0][:], scalar1=sum_h[1][:],
                            scalar2=sum_q[:], op0=ADD, op1=ADD)
    nc.vector.tensor_scalar(out=ssqs_c[:], in0=ssq_a[:], scalar1=ssq_b[:],
                            scalar2=ssq_c2[:], op0=ADD, op1=ADD)
    nc.vector.tensor_scalar(out=msq_c[:], in0=m_c[:], scalar1=m_c[:],
                            scalar2=1.0 / (D * D), op0=MULT, op1=MULT)
    nc.vector.scalar_tensor_tensor(
        out=var_c[:], in0=ssqs_c[:], scalar=1.0 / D, in1=msq_c[:],
        op0=MULT, op1=SUB)
    nc.scalar.activation(std_c[:], var_c[:], SQRT, bias=eps_c[:], scale=1.0)
    nc.vector.reciprocal(istd_c[:], std_c[:])
    nc.vector.tensor_scalar(out=nmi_c[:], in0=m_c[:], scalar1=istd_c[:],
                            scalar2=-1.0 / D, op0=MULT, op1=MULT)

    for h in range(NH):
        sl = slice(512 * h, 512 * (h + 1))
        if general_ln:
            # out = hpre*istd*lns + (lnb - m*istd*lns)
            nc.vector.scalar_tensor_tensor(
                out=t2_sb[:, sl], in0=lnsr_sb[:, sl], scalar=nmi_c[:],
                in1=lnbr_sb[:, sl], op0=MULT, op1=ADD)
            nc.vector.scalar_tensor_tensor(
                out=y_sb[:, sl], in0=hpre_sb[:, sl], scalar=istd_c[:],
                in1=lnsr_sb[:, sl], op0=MULT, op1=MULT)
            nc.vector.tensor_add(out_sb[:, sl], y_sb[:, sl], t2_sb[:, sl])
        else:
            # ln_scale==1, ln_bias==0: out = hpre*istd - m*istd
            nc.vector.tensor_scalar(
                out=out_sb[:, sl], in0=hpre_sb[:, sl],
                scalar1=istd_c[:], scalar2=nmi_c[:], op0=MULT, op1=ADD)
        nc.sync.dma_start(out=out_d.ap()[:, sl], in_=out_sb[:, sl])


def _dr_layout(m, scale):
    """[1024 k, 1024 out] f32 -> [128, (h j i n)] fp8 DoubleRow layout."""
    a = np.asarray(m * scale, dtype=NP_F8)
    # k -> (j, i, p), out -> (h, n); final [p, h, j, i, n]
    a = a.reshape(JT, 2, 128, NH, 512).transpose(2, 3, 0, 1, 4)
    return np.ascontiguousarray(a.reshape(128, KT * D))


def _prep_in_maps(inputs, general_ln):
    def f32c(x):
        return np.ascontiguousarray(np.asarray(x, dtype=np.float32))

    h_a = f32c(inputs["h_A"])
    alpha = f32c(inputs["alpha"])
    pool = np.asarray(inputs["pool_vectors"], dtype=np.float32)
    w_base = np.asarray(inputs["W_base"], dtype=np.float32)

    # pool_vectors rows: [U_n (D*R) | V_n (R*D) | bias_n (D)]
    u = pool[:, :D * R_RANK].reshape(N_EXP, D, R_RANK)
    v = pool[:, D * R_RANK:2 * D * R_RANK].reshape(N_EXP, R_RANK, D)
    bias_pool = pool[:, 2 * D * R_RANK:]                    # [64, D]
    bb = np.asarray(inputs["b_base"], dtype=np.float32).reshape(1, D)
    # fp8 weights are scaled x32; alpha carries 1/32, so the bias rows
    # need x(32*32) for the pool part and x32 for b_base
    bp = np.concatenate([bias_pool * (WSC * WSC), bb * WSC], axis=0)
    vt = _dr_layout(v.reshape(N_EXP * R_RANK, D).T, WSC)   # [a, (n,r)]
    wt = _dr_layout(w_base.T, WSC)                          # [a, c]
    ut = _dr_layout(u.transpose(0, 2, 1).reshape(N_EXP * R_RANK, D), WSC)
    gm = float(np.asarray(inputs["gamma"], dtype=np.float32)) / WSC
    al_s = alpha / WSC

    in_maps = []
    for k in range(N_CORES):
        rows = slice(B_CORE * k, B_CORE * (k + 1))
        xt = h_a[rows].T                                    # [1024, 32]
        x8 = np.asarray(xt.reshape(JT, 2, 128, B_CORE).transpose(2, 0, 1, 3)
                        .reshape(128, XW), dtype=NP_F8)
        xv = np.concatenate([x8, vt], axis=1)               # [128, 8448]
        fb = np.concatenate(
            [h_a[rows], al_s[rows],
             np.full((B_CORE, 1), gm, np.float32)], axis=1)
        alt = np.concatenate(
            [al_s[rows], np.ones((B_CORE, 1), np.float32)], axis=1).T
        bbb = np.concatenate([bp, alt], axis=1)             # [65, 1056]
        im = {
            "xv": np.ascontiguousarray(xv),
            "fb": f32c(fb),
            "bb": np.ascontiguousarray(np.asarray(bbb, dtype=NP_BF16)),
            "wt": wt, "ut": ut,
        }
        if general_ln:
            im["lns"] = f32c(inputs["ln_scale"]).reshape(1, D)
            im["lnb"] = f32c(inputs["ln_bias"]).reshape(1, D)
        in_maps.append(im)
    return in_maps


def _is_general_ln(inputs):
    lns = np.asarray(inputs["ln_scale"], dtype=np.float32)
    lnb = np.asarray(inputs["ln_bias"], dtype=np.float32)
    return not (np.all(lns == 1.0) and np.all(lnb == 0.0))


def get_compiled(general_ln=False):
    key = bool(general_ln)
    if key not in _COMPILED:
        _COMPILED[key] = _build(key)
    return _COMPILED[key]


def kernel(**inputs):
    general_ln = _is_general_ln(inputs)
    nc = get_compiled(general_ln)
    in_maps = _prep_in_maps(inputs, general_ln)
    res = bass_utils.run_bass_kernel_spmd(
        nc, in_maps, core_ids=list(range(N_CORES)))
    return np.concatenate([r["out"] for r in res.results], axis=0)


# revision 11
# speedup vs baseline: 1.8533x; 1.0226x over previous
"""Trainium2 Bass kernel for the DWA middle layer (moe_routing).

Math (factored form of the reference; W_assembled is never materialized):
    t     = h_A @ V_flat^T                      # [B, N*R]
    s     = t * repeat(alpha, R, axis=1)        # [B, N*R]
    h_T   = s @ U_flat^T + h_A @ W_base^T + [alpha, 1] @ [bias_pool; b_base]
    out   = LayerNorm(h_A + gamma * h_T) * ln_scale + ln_bias

Sharding: data-parallel over the batch dim (32 rows per core, 8 cores);
weights replicated.  The kernel is HBM-bound on the three 1024x1024
weight streams, so they are sent as fp8 e4m3 (scaled x32 on the host;
the scale is folded into alpha, the bias matrix and gamma, so the
device math is unchanged up to fp8 rounding — measured end-to-end
rel-err ~2e-3 against the fp32 reference, well inside the 2e-2 gate).
Matmuls run in DoubleRow fp8 mode (256-deep contraction per pass,
~512 PE cycles per [256k x 32m x 512n] instruction).

Perf notes (from perfetto/NTFF analysis of earlier revisions):
  - A stream of dummy fp8 matmuls at kernel start keeps the PE HAM
    activity window busy so real matmuls run at 2.4 GHz, not the
    1.2 GHz cold clock.
  - The NEFF exit protocol drains every allocated DMA queue ring
    (~115ns x 16 rings per issuing engine), so ALL loads ride one
    HWDGE ring (sync) — the SDMA engines already round-robin between
    queues, so a second ring adds no aggregate bandwidth, only tail.
  - Per-DMA fixed overhead is ~1.3us, so small inputs are packed into
    few blob DMAs and h_A^T(fp8) is concatenated with the V matrix.
  - Host-side prep only re-lays-out / scales / casts inputs; all
    arithmetic between tensors runs on device.
"""

import os
from contextlib import ExitStack

import ml_dtypes
import numpy as np

import concourse.bacc as bacc
import concourse.mybir as mybir
import concourse.tile as tile
from concourse import bass_utils, masks

F32 = mybir.dt.float32
BF16 = mybir.dt.bfloat16
F8 = mybir.dt.float8e4
NP_F8 = ml_dtypes.float8_e4m3
NP_BF16 = ml_dtypes.bfloat16

D = 1024          # d_A == d_B
B_CORE = 32       # batch rows per core
N_EXP = 64        # experts
R_RANK = 16       # rank per expert
N_CORES = 8
KT = 8            # 128-deep contraction tiles
JT = 4            # DoubleRow 256-deep contraction tiles
NH = 2            # output halves of 512
WSC = 32.0        # fp8 weight scale (folded into alpha/bias/gamma)
XW = JT * 2 * B_CORE  # 256 columns of h_A^T tiles
N_WU = int(os.environ.get("DWA_WARMUP_MM", "11"))  # PE warm-up matmuls

_COMPILED = {}


def _build(general_ln):
    nc = bacc.Bacc("TRN2", debug=False, num_devices=N_CORES,
                   enable_partition_id=False)

    # [128, 256 (h_A^T fp8 tiles) | 8192 (V, DoubleRow layout)]
    xv_d = nc.dram_tensor("xv", [128, XW + KT * D], F8, kind="ExternalInput")
    wt_d = nc.dram_tensor("wt", [128, KT * D], F8, kind="ExternalInput")
    ut_d = nc.dram_tensor("ut", [128, KT * D], F8, kind="ExternalInput")
    # bf16 blob: [32, 1024 (h_A) | 64 (alpha/32) | 1 (gamma/32)]
    fb_d = nc.dram_tensor("fb", [B_CORE, D + N_EXP + 1], F32,
                          kind="ExternalInput")
    # bf16 blob: [65, 1024 (bias rows) | 32 (alphaT/32 with ones row)]
    bb_d = nc.dram_tensor("bb", [N_EXP + 1, D + B_CORE], BF16,
                          kind="ExternalInput")
    if general_ln:
        lns_d = nc.dram_tensor("lns", [1, D], F32, kind="ExternalInput")
        lnb_d = nc.dram_tensor("lnb", [1, D], F32, kind="ExternalInput")
    else:
        lns_d = lnb_d = None
    out_d = nc.dram_tensor("out", [B_CORE, D], F32, kind="ExternalOutput")

    with ExitStack() as ctx:
        tc = ctx.enter_context(tile.TileContext(nc))
        _emit(ctx, tc, general_ln, xv_d, wt_d, ut_d, fb_d, bb_d,
              lns_d, lnb_d, out_d)

    nc.compile()
    return nc


def _emit(ctx, tc, general_ln, xv_d, wt_d, ut_d, fb_d, bb_d,
          lns_d, lnb_d, out_d):
    nc = tc.nc
    MULT = mybir.AluOpType.mult
    ADD = mybir.AluOpType.add
    SUB = mybir.AluOpType.subtract
    SQRT = mybir.ActivationFunctionType.Sqrt
    DR = mybir.MatmulPerfMode.DoubleRow

    wpool = ctx.enter_context(tc.tile_pool(name="weights", bufs=1))
    sm = ctx.enter_context(tc.tile_pool(name="small", bufs=1))
    pp = ctx.enter_context(tc.tile_pool(name="psum", bufs=1, space="PSUM"))

    xv_sb = wpool.tile([128, XW + KT * D], F8, tag="xv")
    x8_sb = xv_sb[:, :XW]
    vt_sb = xv_sb[:, XW:]
    wt_sb = wpool.tile([128, KT * D], F8, tag="wt")
    ut_sb = wpool.tile([128, KT * D], F8, tag="ut")

    fb_sb = sm.tile([B_CORE, D + N_EXP + 1], F32, tag="fb")
    ha_sb = fb_sb[:, :D]
    al_sb = fb_sb[:, D:D + N_EXP]
    gmc_sb = fb_sb[:, D + N_EXP:D + N_EXP + 1]
    bb_sb = sm.tile([N_EXP + 1, D + B_CORE], BF16, tag="bb")
    bp_sb = bb_sb[:, :D]
    alt_sb = bb_sb[:, D:D + B_CORE]
    wu_sb = sm.tile([128, 2 * B_CORE + 1024], F8, tag="wu")
    ident = sm.tile([B_CORE, B_CORE], F32, tag="ident")
    s_sb = sm.tile([B_CORE, D], F32, tag="s")
    st_sb = sm.tile([128, KT * B_CORE], F8, tag="st")    # s^T tiles
    hpre_sb = sm.tile([B_CORE, D], F32, tag="hpre")
    sq_sb = sm.tile([B_CORE, D], F32, tag="sq")
    out_sb = sm.tile([B_CORE, D], F32, tag="out")
    sum_h = [sm.tile([B_CORE, 1], F32, tag=f"sumh{h}", name=f"sumh{h}")
             for h in range(NH)]
    sum_q = sm.tile([B_CORE, 1], F32, tag="sumq")
    ssq_a = sm.tile([B_CORE, 1], F32, tag="ssqa")
    ssq_b = sm.tile([B_CORE, 1], F32, tag="ssqb")
    ssq_c2 = sm.tile([B_CORE, 1], F32, tag="ssqc2")
    m_c = sm.tile([B_CORE, 1], F32, tag="mc")
    ssqs_c = sm.tile([B_CORE, 1], F32, tag="ssqsc")
    msq_c = sm.tile([B_CORE, 1], F32, tag="msqc")
    var_c = sm.tile([B_CORE, 1], F32, tag="varc")
    std_c = sm.tile([B_CORE, 1], F32, tag="stdc")
    istd_c = sm.tile([B_CORE, 1], F32, tag="istdc")
    nmi_c = sm.tile([B_CORE, 1], F32, tag="nmic")
    eps_c = sm.tile([B_CORE, 1], F32, tag="epsc")
    warm_c = sm.tile([B_CORE, 1], F32, tag="warmc")
    if general_ln:
        lnsr_sb = sm.tile([B_CORE, D], F32, tag="lnsr")
        lnbr_sb = sm.tile([B_CORE, D], F32, tag="lnbr")
        y_sb = sm.tile([B_CORE, D], F32, tag="y")
        t2_sb = sm.tile([B_CORE, D], F32, tag="t2")

    # ---- loads split over BOTH HWDGE rings in strict consumption
    # order, balanced bytes; U (the accumulation closer) lands last ----
    HH = JT * D
    nc.sync.dma_start(out=xv_sb[:, :XW + HH], in_=xv_d.ap()[:, :XW + HH])
    nc.scalar.dma_start(out=fb_sb[:], in_=fb_d.ap())
    nc.scalar.dma_start(out=xv_sb[:, XW + HH:], in_=xv_d.ap()[:, XW + HH:])
    nc.sync.dma_start(out=bb_sb[:], in_=bb_d.ap())
    nc.sync.dma_start(out=wt_sb[:, :HH], in_=wt_d.ap()[:, :HH])
    nc.scalar.dma_start(out=wt_sb[:, HH:], in_=wt_d.ap()[:, HH:])
    nc.sync.dma_start(out=ut_sb[:, :HH], in_=ut_d.ap()[:, :HH])
    nc.scalar.dma_start(out=ut_sb[:, HH:], in_=ut_d.ap()[:, HH:])
    if general_ln:
        nc.sync.dma_start(out=lnsr_sb[:],
                          in_=lns_d.ap().broadcast_to([B_CORE, D]))
        nc.scalar.dma_start(out=lnbr_sb[:],
                          in_=lnb_d.ap().broadcast_to([B_CORE, D]))

    nc.vector.memset(eps_c[:], 1e-5)
    nc.vector.memset(wu_sb[:], 0.25)
    masks.make_identity(nc, ident[:])
    # preload both ACT tables (Square, Sqrt) off the critical path
    nc.scalar.activation(warm_c[:], eps_c[:],
                         mybir.ActivationFunctionType.Square)
    nc.scalar.activation(warm_c[:], eps_c[:], SQRT, bias=eps_c[:], scale=1.0)

    def dr_view(ap):
        return ap.rearrange("p (two n) -> p two n", two=2)

    def dr_rhs(w_sb, h, j):
        off = h * (JT * D) + j * D
        return dr_view(w_sb[:, off:off + D])

    def dr_lhs(x_sb, j):
        off = j * 2 * B_CORE
        return dr_view(x_sb[:, off:off + 2 * B_CORE])

    t_ps = [pp.tile([B_CORE, 512], F32, tag=f"t{h}", name=f"t_ps{h}")
            for h in range(NH)]
    h_ps = [pp.tile([B_CORE, 512], F32, tag=f"h{h}", name=f"h_ps{h}")
            for h in range(NH)]
    tr_ps = [pp.tile([128, 128], F32, tag=f"tr{h}", name=f"tr_ps{h}")
             for h in range(NH)]
    wu_ps = pp.tile([B_CORE, 512], F32, tag="wu", name="wu_ps")

    # ---- PE warm-up: keep the HAM activity window busy while the
    # first weight chunks stream in, so real matmuls run at 2.4 GHz ----
    wu_lhs = dr_view(wu_sb[:, :2 * B_CORE])
    wu_rhs = dr_view(wu_sb[:, 2 * B_CORE:])
    for i in range(N_WU):
        nc.tensor.matmul(wu_ps[:], wu_lhs, wu_rhs,
                         start=True, stop=True, perf_mode=DR)

    # ---- t = h_A @ V^T ; s = t * repeat(alpha/32, R); s^T tiles ----
    for h in range(NH):
        for j in range(JT):
            nc.tensor.matmul(t_ps[h][:], dr_lhs(x8_sb, j), dr_rhs(vt_sb, h, j),
                             start=(j == 0), stop=(j == JT - 1), perf_mode=DR)
        o3 = s_sb[:, 512 * h:512 * (h + 1)].rearrange(
            "p (n r) -> p n r", r=R_RANK)
        i3 = t_ps[h][:].rearrange("p (n r) -> p n r", r=R_RANK)
        a3 = al_sb[:, 32 * h:32 * (h + 1)].unsqueeze(-1).broadcast_to(
            [B_CORE, 32, R_RANK])
        nc.vector.tensor_mul(o3, i3, a3)
        # four transposes into one PSUM bank, one fp8-casting copy out
        for kk in range(4):
            k = 4 * h + kk
            nc.tensor.transpose(tr_ps[h][:, 32 * kk:32 * (kk + 1)],
                                s_sb[:, 128 * k:128 * (k + 1)], ident[:])
        nc.vector.tensor_copy(st_sb[:, 128 * h:128 * (h + 1)], tr_ps[h][:])

    # ---- 32*h_T = [al/32,1]@bp' + h_A @ (32W)^T + s @ (32U)^T ----
    # U arrives last, so U matmuls close each accumulation group.
    for h in range(NH):
        nc.tensor.matmul(h_ps[h][:], alt_sb[:],
                         bp_sb[:, 512 * h:512 * (h + 1)],
                         start=True, stop=False)
    for h in range(NH):
        for j in range(JT):
            nc.tensor.matmul(h_ps[h][:], dr_lhs(x8_sb, j), dr_rhs(wt_sb, h, j),
                             start=False, stop=False, perf_mode=DR)
    for h in range(NH):
        for j in range(JT):
            nc.tensor.matmul(h_ps[h][:], dr_lhs(st_sb, j), dr_rhs(ut_sb, h, j),
                             start=False, stop=(j == JT - 1), perf_mode=DR)
        if h == 0:
            sl = slice(0, 512)
            # h_pre = (gamma/32)*(32 h_T) + h_A, with row-sums for the mean
            nc.vector.scalar_tensor_tensor(
                out=hpre_sb[:, sl], in0=h_ps[0][:], scalar=gmc_sb,
                in1=ha_sb[:, sl], op0=MULT, op1=ADD,
                accum_out=sum_h[0][:])
            nc.scalar.activation(sq_sb[:, sl], hpre_sb[:, sl],
                                 mybir.ActivationFunctionType.Square,
                                 accum_out=ssq_a[:])
        else:
            # critical-path half: square runs on DVE right behind hpre
            sl = slice(512, 1024)
            nc.vector.scalar_tensor_tensor(
                out=hpre_sb[:, sl], in0=h_ps[1][:], scalar=gmc_sb,
                in1=ha_sb[:, sl], op0=MULT, op1=ADD,
                accum_out=sum_h[1][:])
            nc.vector.scalar_tensor_tensor(
                out=sq_sb[:, sl], in0=hpre_sb[:, sl], scalar=1.0,
                in1=hpre_sb[:, sl], op0=MULT, op1=MULT,
                accum_out=ssq_b[:])

    # ---- LayerNorm via E[x^2] - E[x]^2 ----
    # m_c holds D*mean; the 1/D folds into msq and nmi scalars
    nc.vector.tensor_add(m_c[:], sum_h[0][:], sum_h[1][:])
    nc.vector.tensor_add(ssqs_c[:], ssq_a[:], ssq_b[:])
    nc.vector.tensor_scalar(out=msq_c[:], in0=m_c[:], scalar1=m_c[:],
                            scalar2=1.0 / (D * D), op0=MULT, op1=MULT)
    nc.vector.scalar_tensor_tensor(
        out=var_c[:], in0=ssqs_c[:], scalar=1.0 / D, in1=msq_c[:],
        op0=MULT, op1=SUB)
    nc.scalar.activation(std_c[:], var_c[:], SQRT, bias=eps_c[:], scale=1.0)
    nc.vector.reciprocal(istd_c[:], std_c[:])
    nc.vector.tensor_scalar(out=nmi_c[:], in0=m_c[:], scalar1=istd_c[:],
                            scalar2=-1.0 / D, op0=MULT, op1=MULT)

    for h in range(NH):
        sl = slice(512 * h, 512 * (h + 1))
        if general_ln:
            # out = hpre*istd*lns + (lnb - m*istd*lns)
            nc.vector.scalar_tensor_tensor(
                out=t2_sb[:, sl], in0=lnsr_sb[:, sl], scalar=nmi_c[:],
                in1=lnbr_sb[:, sl], op0=MULT, op1=ADD)
            nc.vector.scalar_tensor_tensor(
                out=y_sb[:, sl], in0=hpre_sb[:, sl], scalar=istd_c[:],
                in1=lnsr_sb[:, sl], op0=MULT, op1=MULT)
            nc.vector.tensor_add(out_sb[:, sl], y_sb[:, sl], t2_sb[:, sl])
        else:
            # ln_scale==1, ln_bias==0: out = hpre*istd - m*istd
            nc.vector.tensor_scalar(
                out=out_sb[:, sl], in0=hpre_sb[:, sl],
                scalar1=istd_c[:], scalar2=nmi_c[:], op0=MULT, op1=ADD)
        nc.sync.dma_start(out=out_d.ap()[:, sl], in_=out_sb[:, sl])


def _dr_layout(m, scale):
    """[1024 k, 1024 out] f32 -> [128, (h j i n)] fp8 DoubleRow layout."""
    a = np.asarray(m * scale, dtype=NP_F8)
    # k -> (j, i, p), out -> (h, n); final [p, h, j, i, n]
    a = a.reshape(JT, 2, 128, NH, 512).transpose(2, 3, 0, 1, 4)
    return np.ascontiguousarray(a.reshape(128, KT * D))


def _prep_in_maps(inputs, general_ln):
    def f32c(x):
        return np.ascontiguousarray(np.asarray(x, dtype=np.float32))

    h_a = f32c(inputs["h_A"])
    alpha = f32c(inputs["alpha"])
    pool = np.asarray(inputs["pool_vectors"], dtype=np.float32)
    w_base = np.asarray(inputs["W_base"], dtype=np.float32)

    # pool_vectors rows: [U_n (D*R) | V_n (R*D) | bias_n (D)]
    u = pool[:, :D * R_RANK].reshape(N_EXP, D, R_RANK)
    v = pool[:, D * R_RANK:2 * D * R_RANK].reshape(N_EXP, R_RANK, D)
    bias_pool = pool[:, 2 * D * R_RANK:]                    # [64, D]
    bb = np.asarray(inputs["b_base"], dtype=np.float32).reshape(1, D)
    # fp8 weights are scaled x32; alpha carries 1/32, so the bias rows
    # need x(32*32) for the pool part and x32 for b_base
    bp = np.concatenate([bias_pool * (WSC * WSC), bb * WSC], axis=0)
    vt = _dr_layout(v.reshape(N_EXP * R_RANK, D).T, WSC)   # [a, (n,r)]
    wt = _dr_layout(w_base.T, WSC)                          # [a, c]
    ut = _dr_layout(u.transpose(0, 2, 1).reshape(N_EXP * R_RANK, D), WSC)
    gm = float(np.asarray(inputs["gamma"], dtype=np.float32)) / WSC
    al_s = alpha / WSC

    in_maps = []
    for k in range(N_CORES):
        rows = slice(B_CORE * k, B_CORE * (k + 1))
        xt = h_a[rows].T                                    # [1024, 32]
        x8 = np.asarray(xt.reshape(JT, 2, 128, B_CORE).transpose(2, 0, 1, 3)
                        .reshape(128, XW), dtype=NP_F8)
        xv = np.concatenate([x8, vt], axis=1)               # [128, 8448]
        fb = np.concatenate(
            [h_a[rows], al_s[rows],
             np.full((B_CORE, 1), gm, np.float32)], axis=1)
        alt = np.concatenate(
            [al_s[rows], np.ones((B_CORE, 1), np.float32)], axis=1).T
        bbb = np.concatenate([bp, alt], axis=1)             # [65, 1056]
        im = {
            "xv": np.ascontiguousarray(xv),
            "fb": f32c(fb),
            "bb": np.ascontiguousarray(np.asarray(bbb, dtype=NP_BF16)),
            "wt": wt, "ut": ut,
        }
        if general_ln:
            im["lns"] = f32c(inputs["ln_scale"]).reshape(1, D)
            im["lnb"] = f32c(inputs["ln_bias"]).reshape(1, D)
        in_maps.append(im)
    return in_maps


def _is_general_ln(inputs):
    lns = np.asarray(inputs["ln_scale"], dtype=np.float32)
    lnb = np.asarray(inputs["ln_bias"], dtype=np.float32)
    return not (np.all(lns == 1.0) and np.all(lnb == 0.0))


def get_compiled(general_ln=False):
    key = bool(general_ln)
    if key not in _COMPILED:
        _COMPILED[key] = _build(key)
    return _COMPILED[key]


def kernel(**inputs):
    general_ln = _is_general_ln(inputs)
    nc = get_compiled(general_ln)
    in_maps = _prep_in_maps(inputs, general_ln)
    res = bass_utils.run_bass_kernel_spmd(
        nc, in_maps, core_ids=list(range(N_CORES)))
    return np.concatenate([r["out"] for r in res.results], axis=0)
